# revision 1
# baseline (speedup 1.0000x reference)
"""Trainium2 Bass kernel for nn_D2RLCritic (gnn_message_passing).

Self-contained: kernel(**inputs) takes the FULL unsharded inputs (as from
setup_inputs()) and returns the FULL [256, 1] output, running an SPMD Bass
kernel across 8 NeuronCores.

"""

import numpy as np
from contextlib import ExitStack

from concourse import bass, bacc, mybir, tile
from concourse.mybir import AluOpType as ALU
from concourse.mybir import ActivationFunctionType as AF

P = 128
dt = mybir.dt
EPS = 1e-5


def _wrap_idxs(flat_idx):
    """int16 idx array wrapped in 16 partitions, replicated to 128.
    flat_idx [n] -> [128, n//16] with iw[p, s] = flat[s*16 + (p%16)]."""
    n = len(flat_idx)
    assert n % 16 == 0
    iw = np.asarray(flat_idx, np.int16).reshape(n // 16, 16).T  # [16, n/16]
    return np.tile(iw, (8, 1))  # [128, n/16]


def _sort_edges(src, dst_rel, ngroups, group_of, nblocks):
    """Sort edges by (block, group); per-(b,g) counts."""
    blk = dst_rel // P
    order = np.lexsort((group_of, blk))
    counts = np.zeros((nblocks, ngroups), np.int64)
    np.add.at(counts, (blk, group_of), 1)
    return src[order], dst_rel[order], counts


def _build_layer(sk, dr, counts, NB, NQ, T, CBL, make_idx):
    """Column layout: chunks of CBL blocks; per chunk: for g in NQ: for bb in
    CBL: T tiles of 128. Returns idxw [128, NB*NQ*T*8] int16, drel [128, ncols]."""
    ncols = NB * NQ * T
    idx_flat = np.zeros(ncols * P, np.int64)
    drel = np.full((P, ncols), -1.0, np.float32)
    starts = np.concatenate([[0], np.cumsum(counts.ravel())])[:-1].reshape(counts.shape)
    for b in range(NB):
        cb, bb = divmod(b, CBL)
        for g in range(counts.shape[1]):
            n_e = int(counts[b, g])
            st = int(starts[b, g])
            for t in range(T):
                coli = cb * (NQ * CBL * T) + g * (CBL * T) + bb * T + t
                a, z = t * P, min((t + 1) * P, n_e)
                if a < n_e:
                    seg = slice(st + a, st + z)
                    idx_flat[coli * P : coli * P + (z - a)] = make_idx(sk[seg])
                    drel[: z - a, coli] = dr[seg] - b * P
    return _wrap_idxs(idx_flat), drel


def build_host_data(x, edge_index, batch, n_cores, G, RS1=25000):
    x = np.ascontiguousarray(np.asarray(x, np.float32))
    src_g = np.asarray(edge_index[0], np.int64)
    dst_g = np.asarray(edge_index[1], np.int64)
    batch = np.asarray(batch, np.int64)
    N, F = x.shape
    assert F == 64 and N % n_cores == 0
    NS = N // n_cores
    NB = (NS + P - 1) // P
    NQ = 4
    RS1 = min(RS1, (N + NQ - 1) // NQ)
    assert RS1 * NQ >= N and RS1 <= 32768
    assert N // 4 <= 32768
    CBL = next(c for c in (7, 5, 4, 3, 2, 1) if NB % c == 0)

    percore = []
    T1 = T2 = 1
    for k in range(n_cores):
        lo = k * NS
        m = (dst_g >= lo) & (dst_g < lo + NS)
        s = src_g[m]
        d = dst_g[m] - lo
        e1 = _sort_edges(s, d, NQ, s // RS1, NB)
        e2 = _sort_edges(s, d, NQ, s % 4, NB)
        T1 = max(T1, int(np.ceil(e1[2].max() / P)))
        T2 = max(T2, int(np.ceil(e2[2].max() / P)))
        percore.append((e1, e2))

    in_maps = []
    for k in range(n_cores):
        lo = k * NS
        (s1, d1, c1), (s2, d2, c2) = percore[k]
        idx1, drel1 = _build_layer(s1, d1, c1, NB, NQ, T1, CBL, lambda s: s % RS1)
        idx2, drel2 = _build_layer(s2, d2, c2, NB, NQ, T2, CBL, lambda s: s // 4)
        grel = np.full((P, NB), -1.0, np.float32)
        nmask = np.zeros((P, NB), np.float32)
        for b in range(NB):
            sz = min(P, NS - b * P)
            grel[:sz, b] = batch[lo + b * P : lo + b * P + sz]
            nmask[:sz, b] = 1.0
        xo = np.zeros((NB * P, F), np.float32)
        xo[:NS] = x[lo : lo + NS]
        in_maps.append(
            dict(xfull=x, xown=xo, idx1=idx1, drel1=drel1, idx2=idx2,
                 drel2=drel2, grel=grel, nmask=nmask)
        )
    cfg = dict(N=N, NS=NS, F=F, G=G, NB=NB, T1=T1, T2=T2, NQ=NQ, RS1=RS1,
               CBL=CBL, n_cores=n_cores)
    return in_maps, cfg


def add_weights(in_maps, inputs):
    f32 = np.float32
    w = {}
    w["w1cat"] = np.concatenate(
        [np.asarray(inputs["w1l"], f32), np.asarray(inputs["w1r"], f32)], axis=0
    )  # [128, 16]
    w["w2cat"] = np.concatenate(
        [np.asarray(inputs["w2l"], f32), np.asarray(inputs["w2r"], f32)], axis=0
    )  # [32, 16]
    for name in ("b1l", "b2l", "g1", "be1"):
        w[name] = np.asarray(inputs[name], f32).reshape(1, 16)
    for name in ("gl1", "bl1", "bW1", "bW2", "bW3"):
        w[name] = np.asarray(inputs[name], f32).reshape(16, 1)
    w["bWf"] = np.asarray(inputs["bWf"], f32).reshape(1, 1)
    for name in ("gl2", "bl2", "gl3", "bl3"):
        v = np.asarray(inputs[name], f32).reshape(32, 1)
        w[name + "a"], w[name + "b"] = v[:16].copy(), v[16:].copy()
    w["W1"] = np.asarray(inputs["W1"], f32)
    w["Wf"] = np.asarray(inputs["Wf"], f32)
    for name in ("W2", "W3"):
        v = np.asarray(inputs[name], f32)
        w[name + "a"], w[name + "b"] = v[:16].copy(), v[16:].copy()
    for m in in_maps:
        m.update(w)
    return in_maps


def build_program(cfg, enable_asserts=False):
    NCORES = cfg["n_cores"]
    N, NS, F, G, NB = cfg["N"], cfg["NS"], cfg["F"], cfg["G"], cfg["NB"]
    T1, T2, NQ, RS1, CBL = cfg["T1"], cfg["T2"], cfg["NQ"], cfg["RS1"], cfg["CBL"]
    GT = (G + P - 1) // P
    NCH = NB // CBL
    f32 = dt.float32
    dbg = cfg.get("debug")

    nc = bacc.Bacc(
        "TRN2", target_bir_lowering=False, debug=False,
        enable_asserts=enable_asserts, num_devices=NCORES,
    )
    RG = [list(range(NCORES))]

    xfull_in = nc.dram_tensor("xfull", [N, F], f32, kind="ExternalInput")
    xown_in = nc.dram_tensor("xown", [NB * P, F], f32, kind="ExternalInput")
    idx1_in = nc.dram_tensor("idx1", [P, NB * NQ * T1 * 8], dt.int16, kind="ExternalInput")
    drel1_in = nc.dram_tensor("drel1", [P, NB * NQ * T1], f32, kind="ExternalInput")
    idx2_in = nc.dram_tensor("idx2", [P, NB * NQ * T2 * 8], dt.int16, kind="ExternalInput")
    drel2_in = nc.dram_tensor("drel2", [P, NB * NQ * T2], f32, kind="ExternalInput")
    grel_in = nc.dram_tensor("grel", [P, NB], f32, kind="ExternalInput")
    nmask_in = nc.dram_tensor("nmask", [P, NB], f32, kind="ExternalInput")
    w1cat_in = nc.dram_tensor("w1cat", [2 * F, 16], f32, kind="ExternalInput")
    w2cat_in = nc.dram_tensor("w2cat", [32, 16], f32, kind="ExternalInput")
    row_ins = {
        name: nc.dram_tensor(name, [1, 16], f32, kind="ExternalInput")
        for name in ("b1l", "b2l", "g1", "be1")
    }
    col_names = ("gl1", "bl1", "bW1", "gl2a", "gl2b", "bl2a", "bl2b",
                 "gl3a", "gl3b", "bl3a", "bl3b", "bW2", "bW3")
    col_ins = {
        name: nc.dram_tensor(name, [16, 1], f32, kind="ExternalInput")
        for name in col_names
    }
    col_ins["bWf"] = nc.dram_tensor("bWf", [1, 1], f32, kind="ExternalInput")
    W_ins = {
        name: nc.dram_tensor(name, [16, shp1], f32, kind="ExternalInput")
        for name, shp1 in (
            ("W1", 16), ("W2a", 16), ("W2b", 16), ("W3a", 16), ("W3b", 16), ("Wf", 1),
        )
    }
    out_t = nc.dram_tensor("out", [1, G], f32, kind="ExternalOutput")
    if dbg:
        dbg_h1 = nc.dram_tensor("dbg_h1", [NS, 16], f32, kind="ExternalOutput")
        dbg_stats = nc.dram_tensor("dbg_stats", [1, 32], f32, kind="ExternalOutput")
        dbg_xe = nc.dram_tensor("dbg_xe", [G, 17], f32, kind="ExternalOutput")
        dbg_agg1 = nc.dram_tensor("dbg_agg1", [NB * P, 65], f32, kind="ExternalOutput")
        dbg_tab2 = nc.dram_tensor("dbg_tab2", [N, 16], f32, kind="ExternalOutput")

    iota128_t = nc.inline_tensor(
        np.broadcast_to(np.arange(P, dtype=np.float32), (P, P)).copy(), "iota128"
    )
    iotag_t = nc.inline_tensor(
        np.broadcast_to(np.arange(G, dtype=np.float32), (P, G)).copy(), "iotag"
    )
    ident_t = nc.inline_tensor(np.eye(P, dtype=np.float32), "ident")

    h1sh = nc.dram_tensor("h1sh", [NS, 16], f32, kind="Internal")
    tab2 = nc.dram_tensor("tab2", [N, 16], f32, kind="Internal", addr_space="Shared")
    stin = nc.dram_tensor("stin", [1, 32], f32, kind="Internal")
    stout = nc.dram_tensor("stout", [1, 32], f32, kind="Internal", addr_space="Shared")
    xein = nc.dram_tensor("xein", [G, 17], f32, kind="Internal")
    xeout = nc.dram_tensor("xeout", [G, 17], f32, kind="Internal", addr_space="Shared")

    with tile.TileContext(nc) as tc, ExitStack() as top:
        persist = top.enter_context(tc.tile_pool(name="persist", bufs=1))
        ppsum = top.enter_context(tc.tile_pool(name="persistps", bufs=1, space="PSUM"))

        iota128_s = persist.tile([P, P], f32)
        nc.sync.dma_start(out=iota128_s[:], in_=iota128_t.ap())
        iotag_s = persist.tile([P, G], f32)
        nc.sync.dma_start(out=iotag_s[:], in_=iotag_t.ap())
        ident_s = persist.tile([P, P], f32)
        nc.sync.dma_start(out=ident_s[:], in_=ident_t.ap())
        drel1_s = persist.tile([P, NB * NQ * T1], f32)
        nc.sync.dma_start(out=drel1_s[:], in_=drel1_in.ap())
        drel2_s = persist.tile([P, NB * NQ * T2], f32)
        nc.sync.dma_start(out=drel2_s[:], in_=drel2_in.ap())
        grel_s = persist.tile([P, NB], f32)
        nc.sync.dma_start(out=grel_s[:], in_=grel_in.ap())
        nmask_s = persist.tile([P, NB], f32)
        nc.sync.dma_start(out=nmask_s[:], in_=nmask_in.ap())
        w1cat_s = persist.tile([2 * F, 16], f32)
        nc.sync.dma_start(out=w1cat_s[:], in_=w1cat_in.ap())
        w2cat_s = persist.tile([32, 16], f32)
        nc.sync.dma_start(out=w2cat_s[:], in_=w2cat_in.ap())
        rows_s = {}
        for name, t in row_ins.items():
            rows_s[name] = persist.tile([1, 16], f32, tag=f"row_{name}", name=f"row_{name}")
            nc.sync.dma_start(out=rows_s[name][:], in_=t.ap())
        cols_s = {}
        for name, t in col_ins.items():
            cols_s[name] = persist.tile(list(t.shape), f32, tag=f"col_{name}", name=f"col_{name}")
            nc.sync.dma_start(out=cols_s[name][:], in_=t.ap())
        Ws_s = {}
        for name, t in W_ins.items():
            Ws_s[name] = persist.tile(list(t.shape), f32, tag=f"W_{name}", name=f"W_{name}")
            nc.sync.dma_start(out=Ws_s[name][:], in_=t.ap())

        ones_row = persist.tile([1, P], f32)
        nc.vector.memset(ones_row[:], 1.0)
        ones_col = persist.tile([P, 1], f32)
        nc.vector.memset(ones_col[:], 1.0)

        h1own = persist.tile([P, NB * 16], f32)

        b1l_t = persist.tile([P, 16], f32, tag="b1l_t")
        b2l_t = persist.tile([P, 16], f32, tag="b2l_t")
        a1_t = persist.tile([P, 16], f32, tag="a1_t")
        c1_t = persist.tile([P, 16], f32, tag="c1_t")

        def bcast16(row_ap, dest, pool):
            pt = pool.tile([P, 16], f32, tag="h1p", name="bc16", bufs=1)
            nc.tensor.matmul(out=pt[:], lhsT=ones_row[:], rhs=row_ap, start=True, stop=True)
            nc.vector.tensor_copy(out=dest, in_=pt[:])

        stats_cm = tc.tile_pool(name="statsps", bufs=1, space="PSUM")
        stats_pool = stats_cm.__enter__()
        stats_ps = stats_pool.tile([1, 32], f32, tag="stats", name="stats")

        # ================= L1 =================
        in_q = [xfull_in.ap()[q * RS1 : min((q + 1) * RS1, N), :] for q in range(NQ)]
        with tc.tile_pool(name="l1", bufs=2) as pl, tc.tile_pool(
            name="l1mt", bufs=4
        ) as pmt, tc.tile_pool(name="l1ep", bufs=3) as pep, tc.tile_pool(
            name="l1agg", bufs=2, space="PSUM"
        ) as psA, tc.tile_pool(name="l1mm", bufs=1, space="PSUM") as psM:
            bcast16(rows_s["b1l"][:], b1l_t[:], psM)
            bcast16(rows_s["b2l"][:], b2l_t[:], psM)
            CT = CBL * T1
            for c in range(NCH):
                idxc = pl.tile([P, NQ * CT * 8], dt.int16, tag="idxc")
                nc.sync.dma_start(
                    out=idxc[:], in_=idx1_in.ap()[:, c * NQ * CT * 8 : (c + 1) * NQ * CT * 8]
                )
                E = pl.tile([P, NQ * CT * F], f32, tag="E")
                for q in range(NQ):
                    nc.gpsimd.dma_gather(
                        out_ap=E[:, q * CT * F : (q + 1) * CT * F].rearrange(
                            "p (c f) -> p c f", f=F
                        ),
                        in_ap=in_q[q],
                        idxs_ap=idxc[:, q * CT * 8 : (q + 1) * CT * 8],
                        num_idxs=CT * P,
                        num_idxs_reg=CT * P,
                        elem_size=F,
                        single_packet=False,
                    )
                xog = pl.tile([P, CBL * F], f32, tag="xog")
                nc.sync.dma_start(
                    out=xog[:].rearrange("p (a f) -> p a f", a=CBL),
                    in_=xown_in.ap().rearrange("(cc a p) f -> cc p a f", a=CBL, p=P)[c],
                )
                for bb in range(CBL):
                    b = c * CBL + bb
                    psd = psA.tile([P, 64], f32, tag="aggd", name="aggd")
                    psc = psA.tile([P, 1], f32, tag="aggc", name="aggc")
                    for q in range(NQ):
                        for t in range(T1):
                            j = q * CT + bb * T1 + t
                            col = c * NQ * CT + j
                            MT = pmt.tile([P, P], f32, tag="MT", name="MT")
                            nc.vector.tensor_scalar(
                                out=MT[:], in0=iota128_s[:],
                                scalar1=drel1_s[:, col : col + 1],
                                scalar2=None, op0=ALU.is_equal,
                            )
                            first = q == 0 and t == 0
                            last = q == NQ - 1 and t == T1 - 1
                            nc.tensor.matmul(
                                out=psd[:], lhsT=MT[:],
                                rhs=E[:, j * F : (j + 1) * F],
                                start=first, stop=last, skip_group_check=True,
                            )
                            nc.tensor.matmul(
                                out=psc[:], lhsT=MT[:], rhs=ones_col[:],
                                start=first, stop=last, skip_group_check=True,
                            )
                    if dbg:
                        agd = pep.tile([P, 65], f32, tag="agd", name="agd")
                        nc.vector.tensor_copy(out=agd[:, 0:64], in_=psd[:])
                        nc.vector.tensor_copy(out=agd[:, 64:65], in_=psc[:])
                        nc.sync.dma_start(
                            out=dbg_agg1.ap()[b * P : (b + 1) * P, :], in_=agd[:]
                        )
                    cm = pep.tile([P, 1], f32, tag="cm", name="cm")
                    nc.vector.tensor_scalar_max(out=cm[:], in0=psc[:], scalar1=1.0)
                    inv = pep.tile([P, 1], f32, tag="inv", name="inv")
                    nc.vector.reciprocal(out=inv[:], in_=cm[:])
                    cat = pep.tile([P, 2 * F], f32, tag="cat", name="cat")
                    nc.vector.tensor_scalar(
                        out=cat[:, 0:F], in0=psd[:], scalar1=inv[:],
                        scalar2=None, op0=ALU.mult,
                    )
                    nc.vector.tensor_copy(out=cat[:, F : 2 * F], in_=xog[:, bb * F : (bb + 1) * F])
                    catT_p = psM.tile([2 * F, P], f32, tag="catT", name="catT", bufs=1)
                    nc.tensor.transpose(out=catT_p[:], in_=cat[:], identity=ident_s[:])
                    catT_s = pep.tile([2 * F, P], f32, tag="catTs", name="catTs")
                    nc.vector.tensor_copy(out=catT_s[:], in_=catT_p[:])
                    h1p = psM.tile([P, 16], f32, tag="h1p", name="h1p", bufs=1)
                    nc.tensor.matmul(
                        out=h1p[:], lhsT=catT_s[:], rhs=w1cat_s[:], start=True, stop=True
                    )
                    h1b = pep.tile([P, 16], f32, tag="h1b", name="h1b")
                    nc.vector.tensor_tensor(out=h1b[:], in0=h1p[:], in1=b1l_t[:], op=ALU.add)
                    nc.scalar.activation(out=h1b[:], in_=h1b[:], func=AF.Relu)
                    sz = min(P, NS - b * P)
                    nc.sync.dma_start(out=h1sh.ap()[b * P : b * P + sz, :], in_=h1b[:sz, :])
                    nc.vector.tensor_copy(out=h1own[:, b * 16 : (b + 1) * 16], in_=h1b[:])
                    sq = pep.tile([P, 32], f32, tag="sq", name="sq")
                    nc.vector.tensor_copy(out=sq[:, 0:16], in_=h1b[:])
                    nc.scalar.square(out=sq[:, 16:32], in_=h1b[:])
                    nc.tensor.matmul(
                        out=stats_ps[:], lhsT=nmask_s[:, b : b + 1], rhs=sq[:],
                        start=(b == 0), stop=(b == NB - 1), skip_group_check=True,
                    )
        nc.gpsimd.collective_compute(
            "AllGather", ALU.bypass, replica_groups=RG,
            ins=[h1sh.ap()], outs=[tab2.ap()],
        )
        with tc.tile_pool(name="st", bufs=1) as pst:
            sts = pst.tile([1, 32], f32)
            nc.vector.tensor_copy(out=sts[:], in_=stats_ps[:])
            nc.sync.dma_start(out=stin.ap(), in_=sts[:])
        stats_cm.__exit__(None, None, None)
        nc.gpsimd.collective_compute(
            "AllReduce", ALU.add, replica_groups=RG,
            ins=[stin.ap()], outs=[stout.ap()],
        )
        if dbg:
            nc.sync.dma_start(out=dbg_h1.ap(), in_=h1sh.ap())
            nc.sync.dma_start(out=dbg_stats.ap(), in_=stout.ap())
            nc.sync.dma_start(out=dbg_tab2.ap(), in_=tab2.ap())

        # ---- BN affine tiles
        with tc.tile_pool(name="ph3", bufs=1) as pp3, tc.tile_pool(
            name="ph3ps", bufs=1, space="PSUM"
        ) as ps3:
            st = pp3.tile([1, 32], f32)
            nc.sync.dma_start(out=st[:], in_=stout.ap())
            mu = pp3.tile([1, 16], f32, tag="mu")
            nc.vector.tensor_scalar(
                out=mu[:], in0=st[:, 0:16], scalar1=1.0 / N, scalar2=None, op0=ALU.mult
            )
            var = pp3.tile([1, 16], f32, tag="var")
            nc.vector.tensor_scalar(
                out=var[:], in0=st[:, 16:32], scalar1=1.0 / N, scalar2=None, op0=ALU.mult
            )
            musq = pp3.tile([1, 16], f32, tag="musq")
            nc.vector.tensor_tensor(out=musq[:], in0=mu[:], in1=mu[:], op=ALU.mult)
            nc.vector.tensor_tensor(out=var[:], in0=var[:], in1=musq[:], op=ALU.subtract)
            nc.vector.tensor_scalar(
                out=var[:], in0=var[:], scalar1=EPS, scalar2=None, op0=ALU.add
            )
            sd = pp3.tile([1, 16], f32, tag="sd")
            nc.scalar.sqrt(out=sd[:], in_=var[:])
            rstd = pp3.tile([1, 16], f32, tag="rstd")
            nc.vector.reciprocal(out=rstd[:], in_=sd[:])
            a1r = pp3.tile([1, 16], f32, tag="a1r")
            nc.vector.tensor_tensor(out=a1r[:], in0=rows_s["g1"][:], in1=rstd[:], op=ALU.mult)
            c1r = pp3.tile([1, 16], f32, tag="c1r")
            nc.vector.tensor_tensor(out=c1r[:], in0=a1r[:], in1=mu[:], op=ALU.mult)
            nc.vector.tensor_tensor(
                out=c1r[:], in0=rows_s["be1"][:], in1=c1r[:], op=ALU.subtract
            )
            bcast16(a1r[:], a1_t[:], ps3)
            bcast16(c1r[:], c1_t[:], ps3)

        # ================= L2 =================
        ro_pool = top.enter_context(tc.tile_pool(name="rops", bufs=1, space="PSUM"))
        ro_ps = [
            ro_pool.tile([min(P, G - gt * P), 17], f32, tag=f"ro{gt}", name=f"ro{gt}")
            for gt in range(GT)
        ]
        tab2r = tab2.ap().rearrange("(a b) f -> a (b f)", b=4)  # [N/4, 64]
        with tc.tile_pool(name="l2", bufs=2) as pl, tc.tile_pool(
            name="l2mt", bufs=4
        ) as pmt, tc.tile_pool(name="l2ep", bufs=3) as pep, tc.tile_pool(
            name="l2agg", bufs=2, space="PSUM"
        ) as psA, tc.tile_pool(name="l2mm", bufs=1, space="PSUM") as psM:
            CT = CBL * T2
            for c in range(NCH):
                idxc = pl.tile([P, NQ * CT * 8], dt.int16, tag="idxc")
                nc.sync.dma_start(
                    out=idxc[:], in_=idx2_in.ap()[:, c * NQ * CT * 8 : (c + 1) * NQ * CT * 8]
                )
                E = pl.tile([P, NQ * CT * F], f32, tag="E")
                for q in range(NQ):
                    nc.gpsimd.dma_gather(
                        out_ap=E[:, q * CT * F : (q + 1) * CT * F].rearrange(
                            "p (c f) -> p c f", f=F
                        ),
                        in_ap=tab2r,
                        idxs_ap=idxc[:, q * CT * 8 : (q + 1) * CT * 8],
                        num_idxs=CT * P,
                        num_idxs_reg=CT * P,
                        elem_size=F,
                        single_packet=False,
                    )
                for bb in range(CBL):
                    b = c * CBL + bb
                    psd = psA.tile([P, 16], f32, tag="aggd2", name="aggd2")
                    psc = psA.tile([P, 1], f32, tag="aggc2", name="aggc2")
                    for q in range(NQ):
                        for t in range(T2):
                            j = q * CT + bb * T2 + t
                            col = c * NQ * CT + j
                            MT = pmt.tile([P, P], f32, tag="MT", name="MT")
                            nc.vector.tensor_scalar(
                                out=MT[:], in0=iota128_s[:],
                                scalar1=drel2_s[:, col : col + 1],
                                scalar2=None, op0=ALU.is_equal,
                            )
                            first = q == 0 and t == 0
                            last = q == NQ - 1 and t == T2 - 1
                            nc.tensor.matmul(
                                out=psd[:], lhsT=MT[:],
                                rhs=E[:, j * F + q * 16 : j * F + q * 16 + 16],
                                start=first, stop=last, skip_group_check=True,
                            )
                            nc.tensor.matmul(
                                out=psc[:], lhsT=MT[:], rhs=ones_col[:],
                                start=first, stop=last, skip_group_check=True,
                            )
                    cm = pep.tile([P, 1], f32, tag="cm", name="cm")
                    nc.vector.tensor_scalar_max(out=cm[:], in0=psc[:], scalar1=1.0)
                    inv = pep.tile([P, 1], f32, tag="inv", name="inv")
                    nc.vector.reciprocal(out=inv[:], in_=cm[:])
                    msk = pep.tile([P, 1], f32, tag="msk", name="msk")
                    nc.vector.tensor_scalar_min(out=msk[:], in0=psc[:], scalar1=1.0)
                    cat = pep.tile([P, 32], f32, tag="cat2", name="cat2")
                    nc.vector.tensor_scalar(
                        out=cat[:, 0:16], in0=psd[:], scalar1=inv[:],
                        scalar2=None, op0=ALU.mult,
                    )
                    nc.vector.tensor_tensor(out=cat[:, 0:16], in0=cat[:, 0:16], in1=a1_t[:], op=ALU.mult)
                    ct = pep.tile([P, 16], f32, tag="ct", name="ct")
                    nc.vector.tensor_scalar(
                        out=ct[:], in0=c1_t[:], scalar1=msk[:], scalar2=None, op0=ALU.mult
                    )
                    nc.vector.tensor_tensor(out=cat[:, 0:16], in0=cat[:, 0:16], in1=ct[:], op=ALU.add)
                    nc.vector.tensor_tensor(
                        out=cat[:, 16:32], in0=h1own[:, b * 16 : (b + 1) * 16],
                        in1=a1_t[:], op=ALU.mult,
                    )
                    nc.vector.tensor_tensor(
                        out=cat[:, 16:32], in0=cat[:, 16:32], in1=c1_t[:], op=ALU.add
                    )
                    catT_p = psM.tile([32, P], f32, tag="catT2", name="catT2", bufs=1)
                    nc.tensor.transpose(out=catT_p[:], in_=cat[:], identity=ident_s[:])
                    catT_s = pep.tile([32, P], f32, tag="catTs2", name="catTs2")
                    nc.vector.tensor_copy(out=catT_s[:], in_=catT_p[:])
                    h2p = psM.tile([P, 16], f32, tag="h2p", name="h2p", bufs=1)
                    nc.tensor.matmul(
                        out=h2p[:], lhsT=catT_s[:], rhs=w2cat_s[:], start=True, stop=True
                    )
                    h2e = pep.tile([P, 17], f32, tag="h2e", name="h2e")
                    nc.vector.tensor_tensor(out=h2e[:, 0:16], in0=h2p[:], in1=b2l_t[:], op=ALU.add)
                    nc.scalar.activation(out=h2e[:, 0:16], in_=h2e[:, 0:16], func=AF.Relu)
                    nc.vector.memset(h2e[:, 16:17], 1.0)
                    MTg = pmt.tile([P, G], f32, tag="MTg", name="MTg")
                    nc.vector.tensor_scalar(
                        out=MTg[:], in0=iotag_s[:], scalar1=grel_s[:, b : b + 1],
                        scalar2=None, op0=ALU.is_equal,
                    )
                    for gt in range(GT):
                        gsz = min(P, G - gt * P)
                        nc.tensor.matmul(
                            out=ro_ps[gt][:], lhsT=MTg[:, gt * P : gt * P + gsz],
                            rhs=h2e[:], start=(b == 0), stop=(b == NB - 1),
                            skip_group_check=True,
                        )

        # ================= readout =================
        with tc.tile_pool(name="ph5", bufs=1) as pp5, tc.tile_pool(
            name="ph5ps", bufs=1, space="PSUM"
        ) as ps5:
            for gt in range(GT):
                gsz = min(P, G - gt * P)
                ro_s = pp5.tile([P, 17], f32, tag=f"ros{gt}", name=f"ros{gt}")
                nc.vector.tensor_copy(out=ro_s[:gsz, :], in_=ro_ps[gt][:])
                nc.sync.dma_start(out=xein.ap()[gt * P : gt * P + gsz, :], in_=ro_s[:gsz, :])
            nc.gpsimd.collective_compute(
                "AllReduce", ALU.add, replica_groups=RG,
                ins=[xein.ap()], outs=[xeout.ap()],
            )
            if dbg:
                nc.sync.dma_start(out=dbg_xe.ap(), in_=xeout.ap())
            xeT = pp5.tile([16, G], f32, tag="xeT")
            for gt in range(GT):
                gsz = min(P, G - gt * P)
                xa = pp5.tile([P, 17], f32, tag=f"xa{gt}", name=f"xa{gt}")
                nc.sync.dma_start(out=xa[:gsz, :], in_=xeout.ap()[gt * P : gt * P + gsz, :])
                cm2 = pp5.tile([P, 1], f32, tag=f"cm2{gt}", name=f"cm2{gt}")
                nc.vector.tensor_scalar_max(out=cm2[:gsz], in0=xa[:gsz, 16:17], scalar1=1.0)
                inv2 = pp5.tile([P, 1], f32, tag=f"inv2{gt}", name=f"inv2{gt}")
                nc.vector.reciprocal(out=inv2[:gsz], in_=cm2[:gsz])
                xe = pp5.tile([P, 16], f32, tag=f"xe{gt}", name=f"xe{gt}")
                nc.vector.tensor_scalar(
                    out=xe[:gsz], in0=xa[:gsz, 0:16], scalar1=inv2[:gsz],
                    scalar2=None, op0=ALU.mult,
                )
                tp = ps5.tile([16, P], f32, tag=f"tp{gt}", name=f"tp{gt}")
                nc.tensor.transpose(out=tp[:, :gsz], in_=xe[:gsz, :], identity=ident_s[:gsz, :gsz])
                nc.vector.tensor_copy(out=xeT[:, gt * P : gt * P + gsz], in_=tp[:, :gsz])

            def bn_t(src_ap, Fd, gl, bl, dest):
                s = pp5.tile([Fd, 1], f32, tag=f"bns{Fd}", name=f"bns{Fd}")
                nc.vector.tensor_reduce(out=s[:], in_=src_ap, axis=mybir.AxisListType.X, op=ALU.add)
                mu5 = pp5.tile([Fd, 1], f32, tag=f"bnmu{Fd}", name=f"bnmu{Fd}")
                nc.vector.tensor_scalar(
                    out=mu5[:], in0=s[:], scalar1=1.0 / G, scalar2=None, op0=ALU.mult
                )
                d = pp5.tile([Fd, G], f32, tag=f"bnd{Fd}", name=f"bnd{Fd}")
                nc.vector.tensor_scalar(
                    out=d[:], in0=src_ap, scalar1=mu5[:], scalar2=None, op0=ALU.subtract
                )
                sq5 = pp5.tile([Fd, G], f32, tag=f"bnsq{Fd}", name=f"bnsq{Fd}")
                nc.vector.tensor_tensor(out=sq5[:], in0=d[:], in1=d[:], op=ALU.mult)
                v = pp5.tile([Fd, 1], f32, tag=f"bnv{Fd}", name=f"bnv{Fd}")
                nc.vector.tensor_reduce(out=v[:], in_=sq5[:], axis=mybir.AxisListType.X, op=ALU.add)
                nc.vector.tensor_scalar(
                    out=v[:], in0=v[:], scalar1=1.0 / G, scalar2=EPS, op0=ALU.mult, op1=ALU.add
                )
                sd5 = pp5.tile([Fd, 1], f32, tag=f"bnsd{Fd}", name=f"bnsd{Fd}")
                nc.scalar.sqrt(out=sd5[:], in_=v[:])
                rs5 = pp5.tile([Fd, 1], f32, tag=f"bnrs{Fd}", name=f"bnrs{Fd}")
                nc.vector.reciprocal(out=rs5[:], in_=sd5[:])
                sc5 = pp5.tile([Fd, 1], f32, tag=f"bnsc{Fd}", name=f"bnsc{Fd}")
                nc.vector.tensor_tensor(out=sc5[:], in0=gl, in1=rs5[:], op=ALU.mult)
                nc.vector.tensor_scalar(
                    out=dest, in0=d[:], scalar1=sc5[:], scalar2=bl, op0=ALU.mult, op1=ALU.add
                )

            bn1 = pp5.tile([16, G], f32, tag="bn1")
            bn_t(xeT[:], 16, cols_s["gl1"][:], cols_s["bl1"][:], bn1[:])
            z1p = ps5.tile([16, G], f32, tag="z1p")
            nc.tensor.matmul(out=z1p[:], lhsT=Ws_s["W1"][:], rhs=bn1[:], start=True, stop=True)
            zs1 = pp5.tile([16, G], f32, tag="zs1")
            nc.scalar.activation(out=zs1[:], in_=z1p[:], func=AF.Relu, bias=cols_s["bW1"][:], scale=1.0)
            bn2a = pp5.tile([16, G], f32, tag="bn2a")
            bn_t(zs1[:], 16, cols_s["gl2a"][:], cols_s["bl2a"][:], bn2a[:])
            bn2b = pp5.tile([16, G], f32, tag="bn2b")
            bn_t(xeT[:], 16, cols_s["gl2b"][:], cols_s["bl2b"][:], bn2b[:])
            z2p = ps5.tile([16, G], f32, tag="z2p")
            nc.tensor.matmul(out=z2p[:], lhsT=Ws_s["W2a"][:], rhs=bn2a[:], start=True, stop=False)
            nc.tensor.matmul(out=z2p[:], lhsT=Ws_s["W2b"][:], rhs=bn2b[:], start=False, stop=True)
            zs2 = pp5.tile([16, G], f32, tag="zs2")
            nc.scalar.activation(out=zs2[:], in_=z2p[:], func=AF.Relu, bias=cols_s["bW2"][:], scale=1.0)
            bn3a = pp5.tile([16, G], f32, tag="bn3a")
            bn_t(zs2[:], 16, cols_s["gl3a"][:], cols_s["bl3a"][:], bn3a[:])
            bn3b = pp5.tile([16, G], f32, tag="bn3b")
            bn_t(xeT[:], 16, cols_s["gl3b"][:], cols_s["bl3b"][:], bn3b[:])
            z3p = ps5.tile([16, G], f32, tag="z3p")
            nc.tensor.matmul(out=z3p[:], lhsT=Ws_s["W3a"][:], rhs=bn3a[:], start=True, stop=False)
            nc.tensor.matmul(out=z3p[:], lhsT=Ws_s["W3b"][:], rhs=bn3b[:], start=False, stop=True)
            z3 = pp5.tile([16, G], f32, tag="z3")
            nc.scalar.activation(out=z3[:], in_=z3p[:], func=AF.Relu, bias=cols_s["bW3"][:], scale=1.0)
            ofp = ps5.tile([1, G], f32, tag="ofp")
            nc.tensor.matmul(out=ofp[:], lhsT=Ws_s["Wf"][:], rhs=z3[:], start=True, stop=True)
            ofs = pp5.tile([1, G], f32, tag="ofs")
            nc.vector.tensor_scalar(
                out=ofs[:], in0=ofp[:], scalar1=cols_s["bWf"][:], scalar2=None, op0=ALU.add
            )
            nc.sync.dma_start(out=out_t.ap(), in_=ofs[:])

    nc.compile()
    return nc


def run(inputs, n_cores=8, G=256, cfg_overrides=None, trace=False, enable_asserts=False):
    from concourse.bass_utils import run_bass_kernel_spmd

    in_maps, cfg = build_host_data(
        inputs["x"], inputs["edge_index"], inputs["batch"], n_cores, G
    )
    if cfg_overrides:
        cfg.update(cfg_overrides)
    add_weights(in_maps, inputs)
    nc = build_program(cfg, enable_asserts=enable_asserts)
    res = run_bass_kernel_spmd(nc, in_maps, core_ids=list(range(n_cores)), trace=trace)
    out = res.results[0]["out"].reshape(G, 1)
    return out, res, cfg


def kernel(**inputs):
    """Full inputs -> full [256, 1] output. Shards internally across 8 cores."""
    from concourse.bass_utils import run_bass_kernel_spmd

    n_cores = 8
    G = 256
    in_maps, cfg = build_host_data(
        inputs["x"], inputs["edge_index"], inputs["batch"], n_cores, G
    )
    add_weights(in_maps, inputs)
    nc = build_program(cfg, enable_asserts=False)
    res = run_bass_kernel_spmd(nc, in_maps, core_ids=list(range(n_cores)))
    out = res.results[0]["out"].reshape(G, 1).astype(np.float32)
    return out



# revision 8
# speedup vs baseline: 3.8140x; 3.8140x over previous
"""Trainium2 Bass kernel for nn_D2RLCritic (gnn_message_passing).

Self-contained: kernel(**inputs) takes the FULL unsharded inputs (as from
setup_inputs()) and returns the FULL [256, 1] output, running an SPMD Bass
kernel across 8 NeuronCores.

Design: dst-sharded graph (12544 nodes/core, 98 blocks of 128). Per-edge
feature fetches use gpsimd ap_gather from SBUF-resident transposed feature
stripes ([128, 12544]: partition 16r+f = feature f of node range r), with
per-16-partition-group index streams. L1 projects x@w1l per node first, so
both layers gather 16-dim rows. Gathered columns are transposed on PE into
slot-major E tiles; a bf16 one-hot (dst within block) matmul accumulates the
segment sum in PSUM. Degrees/masks are host-precomputed index tables.
"""

import numpy as np
from contextlib import ExitStack

from concourse import bass, bacc, mybir, tile
from concourse.mybir import AluOpType as ALU
from concourse.mybir import ActivationFunctionType as AF

P = 128
NR = 8
dt = mybir.dt
EPS = 1e-5
CW = 4096


def build_host_data(x, edge_index, batch, n_cores, G):
    assert n_cores == NR
    x = np.ascontiguousarray(np.asarray(x, np.float32))
    src_g = np.asarray(edge_index[0], np.int64)
    dst_g = np.asarray(edge_index[1], np.int64)
    batch = np.asarray(batch, np.int64)
    N0, F = x.shape
    NS = ((N0 + NR * P - 1) // (NR * P)) * P  # 12544
    Npad = NS * NR
    NB = NS // P

    xp = np.zeros((Npad, F), np.float32)
    xp[:N0] = x
    deg = np.bincount(dst_g, minlength=Npad).astype(np.int64)
    batchp = np.full(Npad, -1, np.int64)
    batchp[:N0] = batch

    # per-core edge sort and cell counts
    per_s, per_d, per_cnt = [], [], []
    for k in range(NR):
        m = (dst_g >= k * NS) & (dst_g < (k + 1) * NS)
        s = src_g[m]
        d = dst_g[m] - k * NS
        blk = d >> 7
        rng = s // NS
        order = np.lexsort((s, rng, blk))
        s, d, blk, rng = s[order], d[order], blk[order], rng[order]
        cnt = np.zeros((NB, NR), np.int64)
        np.add.at(cnt, (blk, rng), 1)
        per_s.append(s)
        per_d.append(d)
        per_cnt.append(cnt)
    cnts = np.stack(per_cnt)            # [NR_cores, NB, NR]
    W = cnts.max(axis=0)                # [NB, NR]

    S = np.zeros((NB, NR), np.int64)    # stream offset of cell (b, r)
    S[1:] = np.cumsum(W, axis=0)[:-1]
    L_r = S[-1] + W[-1]
    L = int(((L_r.max() + P - 1) // P) * P)
    NCH = (L + CW - 1) // CW

    # pieces: per block, ordered list of (r, window, drel_col); windows are
    # 128-col spans of all 8 streams (one transposed square serves 8 ranges)
    pieces = []
    npiece = 0
    for b in range(NB):
        plist = []
        for r in range(NR):
            a, z = int(S[b, r]), int(S[b, r] + W[b, r])
            for win in range(a // P, (z + P - 1) // P):
                plist.append((r, win, npiece))
                npiece += 1
        plist.sort(key=lambda t: (t[1], t[0]))
        pieces.append(plist)
    NPIECE = npiece
    in_maps = []
    for k in range(NR):
        s, d, cnt = per_s[k], per_d[k], per_cnt[k]
        # cell start offsets in the sorted edge array
        estart = np.concatenate([[0], np.cumsum(cnt.ravel())])[:-1].reshape(NB, NR)
        # gather index streams, wrapped per 16-partition group
        apg = np.zeros((P, L // 16), np.int16)
        dstrel = np.full((NR, L), -1.0, np.float32)
        for r in range(NR):
            stream = np.zeros(L, np.int64)
            for b in range(NB):
                n_e = int(cnt[b, r])
                if n_e:
                    e0 = estart[b, r]
                    stream[S[b, r]: S[b, r] + n_e] = s[e0: e0 + n_e] - r * NS
                    dstrel[r, S[b, r]: S[b, r] + n_e] = d[e0: e0 + n_e] - b * P
            apg[16 * r: 16 * (r + 1), :] = (
                stream.reshape(L // 16, 16).T.astype(np.int16)
            )
        # drel: [128, NPIECE]: per piece (r, win) the dst offset of each
        # window slot within its block, -1 outside this (b, r) cell
        drel = np.full((P, NPIECE), -1.0, np.float32)
        for b in range(NB):
            for (r, win, pc) in pieces[b]:
                a, z = int(S[b, r]), int(S[b, r] + W[b, r])
                lo, hi = win * P, (win + 1) * P
                aa, zz = max(a, lo), min(z, hi)
                col = np.full(P, -1.0, np.float32)
                col[aa - lo: zz - lo] = dstrel[r, aa: zz]
                drel[:, pc] = col
        nodes = np.arange(NS) + k * NS
        invd = (1.0 / np.maximum(deg[nodes], 1)).astype(np.float32)
        nmask = (nodes < N0).astype(np.float32)
        grel = np.where(nodes < N0, batchp[nodes], -1).astype(np.float32)
        in_maps.append(dict(
            xown=xp[k * NS:(k + 1) * NS],
            apgidx=apg,
            drel=drel,
            invd=invd.reshape(NB, P).T.copy(),
            nmask=nmask.reshape(NB, P).T.copy(),
            grel=grel.reshape(NB, P).T.copy(),
        ))

    cfg = dict(N=N0, NS=NS, NB=NB, F=F, G=G, NPIECE=NPIECE, L=L, NCH=NCH,
               pieces=pieces, n_cores=NR)
    return in_maps, cfg


def add_weights(in_maps, inputs):
    f32 = np.float32
    w = {}
    w["w1cat"] = np.concatenate(
        [np.asarray(inputs["w1l"], f32), np.asarray(inputs["w1r"], f32)], axis=1
    )  # [64, 32]
    w["w2l"] = np.asarray(inputs["w2l"], f32)
    w["w2r"] = np.asarray(inputs["w2r"], f32)
    for name in ("b1l", "b2l", "g1", "be1"):
        w[name] = np.asarray(inputs[name], f32).reshape(1, 16)
    for name in ("gl1", "bl1", "bW1", "bW2", "bW3"):
        w[name] = np.asarray(inputs[name], f32).reshape(16, 1)
    w["bWf"] = np.asarray(inputs["bWf"], f32).reshape(1, 1)
    for name in ("gl2", "bl2", "gl3", "bl3"):
        v = np.asarray(inputs[name], f32).reshape(32, 1)
        w[name + "a"], w[name + "b"] = v[:16].copy(), v[16:].copy()
    w["W1"] = np.asarray(inputs["W1"], f32)
    w["Wf"] = np.asarray(inputs["Wf"], f32)
    for name in ("W2", "W3"):
        v = np.asarray(inputs[name], f32)
        w[name + "a"], w[name + "b"] = v[:16].copy(), v[16:].copy()
    for m in in_maps:
        m.update(w)
    return in_maps


def build_program(cfg, enable_asserts=False):
    NCORES = cfg["n_cores"]
    N, NS, NB, F, G = cfg["N"], cfg["NS"], cfg["NB"], cfg["F"], cfg["G"]
    NPIECE, L, NCH = cfg["NPIECE"], cfg["L"], cfg["NCH"]
    pieces = cfg["pieces"]
    GT = (G + P - 1) // P
    f32, bf16 = dt.float32, dt.bfloat16

    nc = bacc.Bacc(
        "TRN2", target_bir_lowering=False, debug=False,
        enable_asserts=enable_asserts, num_devices=NCORES,
    )
    RG = [list(range(NCORES))]

    xown_in = nc.dram_tensor("xown", [NS, F], f32, kind="ExternalInput")
    apg_in = nc.dram_tensor("apgidx", [P, L // 16], dt.int16, kind="ExternalInput")
    drel_in = nc.dram_tensor("drel", [P, NPIECE], f32, kind="ExternalInput")
    invd_in = nc.dram_tensor("invd", [P, NB], f32, kind="ExternalInput")
    nmask_in = nc.dram_tensor("nmask", [P, NB], f32, kind="ExternalInput")
    grel_in = nc.dram_tensor("grel", [P, NB], f32, kind="ExternalInput")
    w1cat_in = nc.dram_tensor("w1cat", [F, 32], f32, kind="ExternalInput")
    w2l_in = nc.dram_tensor("w2l", [16, 16], f32, kind="ExternalInput")
    w2r_in = nc.dram_tensor("w2r", [16, 16], f32, kind="ExternalInput")
    row_ins = {
        name: nc.dram_tensor(name, [1, 16], f32, kind="ExternalInput")
        for name in ("b1l", "b2l", "g1", "be1")
    }
    col_names = ("gl1", "bl1", "bW1", "gl2a", "gl2b", "bl2a", "bl2b",
                 "gl3a", "gl3b", "bl3a", "bl3b", "bW2", "bW3")
    col_ins = {
        name: nc.dram_tensor(name, [16, 1], f32, kind="ExternalInput")
        for name in col_names
    }
    col_ins["bWf"] = nc.dram_tensor("bWf", [1, 1], f32, kind="ExternalInput")
    W_ins = {
        name: nc.dram_tensor(name, [16, shp1], f32, kind="ExternalInput")
        for name, shp1 in (
            ("W1", 16), ("W2a", 16), ("W2b", 16), ("W3a", 16), ("W3b", 16), ("Wf", 1),
        )
    }
    out_t = nc.dram_tensor("out", [1, G], f32, kind="ExternalOutput")
    dbg = cfg.get("debug")
    if dbg:
        dbgy = nc.dram_tensor("dbgy", [P, NS], f32, kind="ExternalOutput")
        dbgst = nc.dram_tensor("dbgst", [1, 32], f32, kind="ExternalOutput")
        dbgh1 = nc.dram_tensor("dbgh1", [P, NB * 16], f32, kind="ExternalOutput")
        dbgxe = nc.dram_tensor("dbgxe", [G, 17], f32, kind="ExternalOutput")
        dbgz = nc.dram_tensor("dbgz", [P, NS], f32, kind="ExternalOutput")
        dbgr2 = nc.dram_tensor("dbgr2", [P, NB * 16], f32, kind="ExternalOutput")

    y1tsh = nc.dram_tensor("y1tsh", [16, NS], f32, kind="Internal")
    y1tall = nc.dram_tensor("y1tall", [P, NS], f32, kind="Internal", addr_space="Shared")
    z1tsh = nc.dram_tensor("z1tsh", [16, NS], f32, kind="Internal")
    z1tall = nc.dram_tensor("z1tall", [P, NS], f32, kind="Internal", addr_space="Shared")
    stin = nc.dram_tensor("stin", [1, 32], f32, kind="Internal")
    stout = nc.dram_tensor("stout", [1, 32], f32, kind="Internal", addr_space="Shared")
    xein = nc.dram_tensor("xein", [G, 17], f32, kind="Internal")
    xeout = nc.dram_tensor("xeout", [G, 17], f32, kind="Internal", addr_space="Shared")

    iota128_t = nc.inline_tensor(
        np.broadcast_to(np.arange(P, dtype=np.float32), (P, P)).copy(), "iota128"
    )
    iotag_t = nc.inline_tensor(
        np.broadcast_to(np.arange(G, dtype=np.float32), (P, G)).copy(), "iotag"
    )
    ident_t = nc.inline_tensor(np.eye(P, dtype=np.float32), "ident")

    with tile.TileContext(nc) as tc, ExitStack() as top:
        persist = top.enter_context(tc.tile_pool(name="persist", bufs=1))

        iota_f = persist.tile([P, P], f32)
        nc.sync.dma_start(out=iota_f[:], in_=iota128_t.ap())
        iotag_s = persist.tile([P, G], f32)
        nc.sync.dma_start(out=iotag_s[:], in_=iotag_t.ap())
        ident_s = persist.tile([P, P], f32)
        nc.sync.dma_start(out=ident_s[:], in_=ident_t.ap())
        apg_s = persist.tile([P, L // 16], dt.int16)
        nc.sync.dma_start(out=apg_s[:], in_=apg_in.ap())
        drel_s = persist.tile([P, NPIECE], f32)
        nc.sync.dma_start(out=drel_s[:], in_=drel_in.ap())
        invd_s = persist.tile([P, NB], f32)
        nc.sync.dma_start(out=invd_s[:], in_=invd_in.ap())
        nmask_s = persist.tile([P, NB], f32)
        nc.sync.dma_start(out=nmask_s[:], in_=nmask_in.ap())
        grel_s = persist.tile([P, NB], f32)
        nc.sync.dma_start(out=grel_s[:], in_=grel_in.ap())
        w1cat_s = persist.tile([F, 32], f32)
        nc.sync.dma_start(out=w1cat_s[:], in_=w1cat_in.ap())
        w2l_s = persist.tile([16, 16], f32)
        nc.sync.dma_start(out=w2l_s[:], in_=w2l_in.ap())
        w2r_s = persist.tile([16, 16], f32)
        nc.sync.dma_start(out=w2r_s[:], in_=w2r_in.ap())
        rows_s = {}
        for name, t in row_ins.items():
            rows_s[name] = persist.tile([1, 16], f32, tag=f"row_{name}", name=f"row_{name}")
            nc.sync.dma_start(out=rows_s[name][:], in_=t.ap())
        cols_s = {}
        for name, t in col_ins.items():
            cols_s[name] = persist.tile(list(t.shape), f32, tag=f"col_{name}", name=f"col_{name}")
            nc.sync.dma_start(out=cols_s[name][:], in_=t.ap())
        Ws_s = {}
        for name, t in W_ins.items():
            Ws_s[name] = persist.tile(list(t.shape), f32, tag=f"W_{name}", name=f"W_{name}")
            nc.sync.dma_start(out=Ws_s[name][:], in_=t.ap())

        iota_b = persist.tile([P, P], bf16)
        nc.vector.tensor_copy(out=iota_b[:], in_=iota_f[:])
        w1cat_b = persist.tile([F, 32], bf16)
        nc.vector.tensor_copy(out=w1cat_b[:], in_=w1cat_s[:])
        ones_row = persist.tile([1, P], f32)
        nc.vector.memset(ones_row[:], 1.0)
        ones1 = persist.tile([1, 1], f32)
        nc.vector.memset(ones1[:], 1.0)

        stripe_s = persist.tile([P, NS], f32)       # y1T then z1T
        ytown = persist.tile([16, NS], f32)         # y1T own, then z1T own
        xrb_own = persist.tile([P, NB * 16], f32)
        h1own = persist.tile([P, NB * 16], f32)
        r2own = persist.tile([P, NB * 16], f32)

        b1l_t = persist.tile([P, 16], f32, tag="b1l_t")
        b2r_t = persist.tile([P, 16], f32, tag="b2r_t")

        def bcast16(row_ap, dest, pool):
            pt = pool.tile([P, 16], f32, tag="bc16", name="bc16", bufs=1)
            nc.tensor.matmul(out=pt[:], lhsT=ones_row[:], rhs=row_ap, start=True, stop=True)
            nc.vector.tensor_copy(out=dest, in_=pt[:])

        # warmup: load ap_gather ucode early
        with tc.tile_pool(name="warm", bufs=1) as wp:
            wi = wp.tile([P, 16], dt.int16)
            nc.gpsimd.memset(wi[:], 0)
            wo = wp.tile([P, 16], f32)
            nc.gpsimd.ap_gather(
                out_ap=wo[:], in_ap=iota_f[:], idxs_ap=wi[:, 0:1],
                channels=P, num_elems=P, d=1, num_idxs=16,
            )

        stats_cm = tc.tile_pool(name="statsps", bufs=1, space="PSUM")
        stats_pool = stats_cm.__enter__()
        stats_ps = stats_pool.tile([1, 32], f32, tag="stats", name="stats")

        # ================= L1 prep: y1T own + xrb =================
        with tc.tile_pool(name="p1", bufs=3) as pl, tc.tile_pool(
            name="p1ps", bufs=2, space="PSUM"
        ) as ps1, tc.tile_pool(name="p1s", bufs=3) as sb1, tc.tile_pool(
            name="p1one", bufs=1, space="PSUM"
        ) as ps1o:
            bcast16(rows_s["b1l"][:], b1l_t[:], ps1o)
            for b in range(NB):
                xb = pl.tile([P, F], f32, tag="xb")
                nc.sync.dma_start(out=xb[:], in_=xown_in.ap()[b * P:(b + 1) * P, :])
                xTp = ps1.tile([F, P], f32, tag="xTp", name="xTp")
                nc.tensor.transpose(out=xTp[:], in_=xb[:], identity=ident_s[:])
                xT_s = sb1.tile([F, P], bf16, tag="xTs")
                nc.vector.tensor_copy(out=xT_s[:], in_=xTp[:])
                y1tp = ps1.tile([16, P], f32, tag="y1tp", name="y1tp")
                nc.tensor.matmul(out=y1tp[:], lhsT=w1cat_b[:, 0:16], rhs=xT_s[:],
                                 start=True, stop=True)
                nc.vector.tensor_copy(out=ytown[:, b * P:(b + 1) * P], in_=y1tp[:])
                xrp = ps1.tile([P, 16], f32, tag="xrp", name="xrp")
                nc.tensor.matmul(out=xrp[:], lhsT=xT_s[:], rhs=w1cat_b[:, 16:32],
                                 start=True, stop=True)
                nc.vector.tensor_tensor(out=xrb_own[:, b * 16:(b + 1) * 16],
                                        in0=xrp[:], in1=b1l_t[:], op=ALU.add)
        nc.sync.dma_start(out=y1tsh.ap(), in_=ytown[:])
        nc.gpsimd.collective_compute(
            "AllGather", ALU.bypass, replica_groups=RG,
            ins=[y1tsh.ap()], outs=[y1tall.ap()],
        )
        nc.sync.dma_start(out=stripe_s[:], in_=y1tall.ap())
        if dbg:
            nc.sync.dma_start(out=dbgy.ap(), in_=y1tall.ap())

        # ================= shared edge-layer emitter =================
        WPC = CW // P  # windows per chunk
        fence_pool = top.enter_context(tc.tile_pool(name="fence", bufs=2))

        def pool_fence():
            """Order later gpsimd work after the stripe/idx loads: gpsimd is
            in-order, and this op's reads are dependency-tracked."""
            fp = fence_pool.tile([1, 4], f32, tag="fence", name="fence")
            nc.gpsimd.tensor_tensor(out=fp[:], in0=stripe_s[0:1, 0:4],
                                    in1=apg_s[0:1, 0:8].bitcast(f32),
                                    op=ALU.add)


        def emit_layer(layer, epilogue):
            """Gather + window transposes + segment-sum; epilogue(b, psd, pool)."""
            pool_fence()
            with tc.tile_pool(name=f"ch{layer}", bufs=3) as chp, tc.tile_pool(
                name=f"sqps{layer}", bufs=3, space="PSUM"
            ) as sqps, tc.tile_pool(name=f"sq{layer}", bufs=8) as sqp, tc.tile_pool(
                name=f"mt{layer}", bufs=4
            ) as mtp, tc.tile_pool(name=f"psd{layer}", bufs=2, space="PSUM") as psdp, \
                 tc.tile_pool(name=f"ep{layer}", bufs=2) as epp:
                chunks = {}
                squares = {}
                next_ch = 0

                def ensure_window(win):
                    nonlocal next_ch
                    if win in squares:
                        return
                    while next_ch <= win // WPC and next_ch < NCH:
                        cw = min(CW, L - next_ch * CW)
                        ct = chp.tile([P, CW], f32, tag="chunk")
                        nc.gpsimd.ap_gather(
                            out_ap=ct[:, 0:cw], in_ap=stripe_s[:],
                            idxs_ap=apg_s[:, next_ch * (CW // 16):
                                          next_ch * (CW // 16) + cw // 16],
                            channels=P, num_elems=NS, d=1, num_idxs=cw,
                        )
                        chunks[next_ch] = ct
                        next_ch += 1
                    cc = (win % WPC) * P
                    sq_ps = sqps.tile([P, P], f32, tag="sqps", name="sqps")
                    nc.tensor.transpose(
                        out=sq_ps[:], in_=chunks[win // WPC][:, cc:cc + P],
                        identity=ident_s[:],
                    )
                    sq = sqp.tile([P, P], bf16, tag="sq")
                    nc.scalar.activation(out=sq[:], in_=sq_ps[:], func=AF.Copy)
                    squares[win] = sq

                for b in range(NB):
                    for (r, win, pc) in pieces[b]:
                        ensure_window(win)
                    psd = psdp.tile([P, 16], f32, tag="psd", name="psd")
                    np_b = len(pieces[b])
                    for i, (r, win, pc) in enumerate(pieces[b]):
                        MT = mtp.tile([P, P], bf16, tag="MT", name="MT")
                        nc.vector.tensor_scalar(
                            out=MT[:], in0=iota_b[:],
                            scalar1=drel_s[:, pc: pc + 1],
                            scalar2=None, op0=ALU.is_equal,
                        )
                        nc.tensor.matmul(
                            out=psd[:], lhsT=MT[:],
                            rhs=squares[win][:, 16 * r: 16 * (r + 1)],
                            start=(i == 0), stop=(i == np_b - 1),
                            skip_group_check=True,
                        )
                    epilogue(b, psd, epp)

        # ================= L1 main =================
        def l1_epilogue(b, psd, epp):
            m1 = epp.tile([P, 16], f32, tag="m1", name="m1")
            nc.vector.tensor_scalar(
                out=m1[:], in0=psd[:], scalar1=invd_s[:, b:b + 1],
                scalar2=None, op0=ALU.mult,
            )
            h1b = epp.tile([P, 16], f32, tag="h1b", name="h1b")
            nc.vector.tensor_tensor(
                out=h1b[:], in0=m1[:], in1=xrb_own[:, b * 16:(b + 1) * 16], op=ALU.add
            )
            nc.scalar.activation(out=h1b[:], in_=h1b[:], func=AF.Relu)
            nc.vector.tensor_copy(out=h1own[:, b * 16:(b + 1) * 16], in_=h1b[:])
            sq = epp.tile([P, 32], f32, tag="sq", name="sq")
            nc.vector.tensor_copy(out=sq[:, 0:16], in_=h1b[:])
            nc.scalar.square(out=sq[:, 16:32], in_=h1b[:])
            nc.tensor.matmul(
                out=stats_ps[:], lhsT=nmask_s[:, b:b + 1], rhs=sq[:],
                start=(b == 0), stop=(b == NB - 1), skip_group_check=True,
            )

        emit_layer(1, l1_epilogue)

        with tc.tile_pool(name="st", bufs=1) as pst:
            sts = pst.tile([1, 32], f32)
            nc.vector.tensor_copy(out=sts[:], in_=stats_ps[:])
            nc.sync.dma_start(out=stin.ap(), in_=sts[:])
        stats_cm.__exit__(None, None, None)
        nc.gpsimd.collective_compute(
            "AllReduce", ALU.add, replica_groups=RG,
            ins=[stin.ap()], outs=[stout.ap()],
        )
        if dbg:
            nc.sync.dma_start(out=dbgst.ap(), in_=stout.ap())
            nc.sync.dma_start(out=dbgh1.ap(), in_=h1own[:])

        # ---- BN affine + L2 weight prep ----
        Wla_b = persist.tile([16, 16], bf16)
        Wra_b = persist.tile([16, 16], bf16)
        cvec1T = persist.tile([16, 1], f32)
        with tc.tile_pool(name="ph3", bufs=1) as pp3, tc.tile_pool(
            name="ph3ps", bufs=1, space="PSUM"
        ) as ps3:
            st = pp3.tile([1, 32], f32)
            nc.sync.dma_start(out=st[:], in_=stout.ap())
            mu = pp3.tile([1, 16], f32, tag="mu")
            nc.vector.tensor_scalar(
                out=mu[:], in0=st[:, 0:16], scalar1=1.0 / N, scalar2=None, op0=ALU.mult
            )
            var = pp3.tile([1, 16], f32, tag="var")
            nc.vector.tensor_scalar(
                out=var[:], in0=st[:, 16:32], scalar1=1.0 / N, scalar2=None, op0=ALU.mult
            )
            musq = pp3.tile([1, 16], f32, tag="musq")
            nc.vector.tensor_tensor(out=musq[:], in0=mu[:], in1=mu[:], op=ALU.mult)
            nc.vector.tensor_tensor(out=var[:], in0=var[:], in1=musq[:], op=ALU.subtract)
            nc.vector.tensor_scalar(
                out=var[:], in0=var[:], scalar1=EPS, scalar2=None, op0=ALU.add
            )
            sd = pp3.tile([1, 16], f32, tag="sd")
            nc.scalar.sqrt(out=sd[:], in_=var[:])
            rstd = pp3.tile([1, 16], f32, tag="rstd")
            nc.vector.reciprocal(out=rstd[:], in_=sd[:])
            a_row = pp3.tile([1, 16], f32, tag="a_row")
            nc.vector.tensor_tensor(out=a_row[:], in0=rows_s["g1"][:], in1=rstd[:], op=ALU.mult)
            c_row = pp3.tile([1, 16], f32, tag="c_row")
            nc.vector.tensor_tensor(out=c_row[:], in0=a_row[:], in1=mu[:], op=ALU.mult)
            nc.vector.tensor_tensor(
                out=c_row[:], in0=rows_s["be1"][:], in1=c_row[:], op=ALU.subtract
            )
            acp = ps3.tile([16, 1], f32, tag="acp", name="acp")
            nc.tensor.matmul(out=acp[:], lhsT=a_row[:], rhs=ones1[:], start=True, stop=True)
            ccp = ps3.tile([16, 1], f32, tag="ccp", name="ccp")
            nc.tensor.matmul(out=ccp[:], lhsT=c_row[:], rhs=ones1[:], start=True, stop=True)
            ac = pp3.tile([16, 2], f32, tag="ac")
            nc.vector.tensor_copy(out=ac[:, 0:1], in_=acp[:])
            nc.vector.tensor_copy(out=ac[:, 1:2], in_=ccp[:])
            Wla = pp3.tile([16, 16], f32, tag="Wla")
            nc.vector.tensor_scalar(
                out=Wla[:], in0=w2l_s[:], scalar1=ac[:, 0:1], scalar2=None, op0=ALU.mult
            )
            nc.vector.tensor_copy(out=Wla_b[:], in_=Wla[:])
            Wra = pp3.tile([16, 16], f32, tag="Wra")
            nc.vector.tensor_scalar(
                out=Wra[:], in0=w2r_s[:], scalar1=ac[:, 0:1], scalar2=None, op0=ALU.mult
            )
            nc.vector.tensor_copy(out=Wra_b[:], in_=Wra[:])
            cv1p = ps3.tile([16, 1], f32, tag="cv1p", name="cv1p")
            nc.tensor.matmul(out=cv1p[:], lhsT=w2l_s[:], rhs=ac[:, 1:2], start=True, stop=True)
            nc.vector.tensor_copy(out=cvec1T[:], in_=cv1p[:])
            r2cp = ps3.tile([1, 16], f32, tag="r2cp", name="r2cp")
            nc.tensor.matmul(out=r2cp[:], lhsT=ac[:, 1:2], rhs=w2r_s[:], start=True, stop=True)
            r2c = pp3.tile([1, 16], f32, tag="r2c")
            nc.vector.tensor_tensor(out=r2c[:], in0=r2cp[:], in1=rows_s["b2l"][:], op=ALU.add)
            bcast16(r2c[:], b2r_t[:], ps3)

        # ---- L2 prep: z1T own + r2 ----
        with tc.tile_pool(name="p2", bufs=3) as pl, tc.tile_pool(
            name="p2ps", bufs=2, space="PSUM"
        ) as ps2, tc.tile_pool(name="p2s", bufs=3) as sb2:
            for b in range(NB):
                h1Tp = ps2.tile([16, P], f32, tag="h1Tp", name="h1Tp")
                nc.tensor.transpose(
                    out=h1Tp[:], in_=h1own[:, b * 16:(b + 1) * 16], identity=ident_s[:]
                )
                h1T_s = sb2.tile([16, P], bf16, tag="h1Ts")
                nc.vector.tensor_copy(out=h1T_s[:], in_=h1Tp[:])
                z1tp = ps2.tile([16, P], f32, tag="z1tp", name="z1tp")
                nc.tensor.matmul(out=z1tp[:], lhsT=Wla_b[:], rhs=h1T_s[:], start=True, stop=True)
                nc.vector.tensor_scalar(
                    out=ytown[:, b * P:(b + 1) * P], in0=z1tp[:],
                    scalar1=cvec1T[:], scalar2=None, op0=ALU.add,
                )
                r2p = ps2.tile([P, 16], f32, tag="r2p", name="r2p")
                nc.tensor.matmul(out=r2p[:], lhsT=h1T_s[:], rhs=Wra_b[:], start=True, stop=True)
                nc.vector.tensor_tensor(
                    out=r2own[:, b * 16:(b + 1) * 16], in0=r2p[:], in1=b2r_t[:], op=ALU.add
                )
        nc.sync.dma_start(out=z1tsh.ap(), in_=ytown[:])
        nc.gpsimd.collective_compute(
            "AllGather", ALU.bypass, replica_groups=RG,
            ins=[z1tsh.ap()], outs=[z1tall.ap()],
        )
        nc.sync.dma_start(out=stripe_s[:], in_=z1tall.ap())
        if dbg:
            nc.sync.dma_start(out=dbgz.ap(), in_=z1tall.ap())
            nc.sync.dma_start(out=dbgr2.ap(), in_=r2own[:])

        # ================= L2 main =================
        ro_pool = top.enter_context(tc.tile_pool(name="rops", bufs=1, space="PSUM"))
        ro_ps = [
            ro_pool.tile([min(P, G - gt * P), 17], f32, tag=f"ro{gt}", name=f"ro{gt}")
            for gt in range(GT)
        ]

        def l2_epilogue(b, psd, epp):
            m2 = epp.tile([P, 16], f32, tag="m2", name="m2")
            nc.vector.tensor_scalar(
                out=m2[:], in0=psd[:], scalar1=invd_s[:, b:b + 1],
                scalar2=None, op0=ALU.mult,
            )
            h2e = epp.tile([P, 17], f32, tag="h2e", name="h2e")
            nc.vector.tensor_tensor(
                out=h2e[:, 0:16], in0=m2[:], in1=r2own[:, b * 16:(b + 1) * 16], op=ALU.add
            )
            nc.scalar.activation(out=h2e[:, 0:16], in_=h2e[:, 0:16], func=AF.Relu)
            nc.vector.memset(h2e[:, 16:17], 1.0)
            MTg = epp.tile([P, G], f32, tag="MTg", name="MTg")
            nc.vector.tensor_scalar(
                out=MTg[:], in0=iotag_s[:], scalar1=grel_s[:, b:b + 1],
                scalar2=None, op0=ALU.is_equal,
            )
            for gt in range(GT):
                gsz = min(P, G - gt * P)
                nc.tensor.matmul(
                    out=ro_ps[gt][:], lhsT=MTg[:, gt * P:gt * P + gsz],
                    rhs=h2e[:], start=(b == 0), stop=(b == NB - 1),
                    skip_group_check=True,
                )

        emit_layer(2, l2_epilogue)

        # ================= readout =================
        with tc.tile_pool(name="ph5", bufs=1) as pp5, tc.tile_pool(
            name="ph5ps", bufs=1, space="PSUM"
        ) as ps5:
            for gt in range(GT):
                gsz = min(P, G - gt * P)
                ro_s = pp5.tile([P, 17], f32, tag=f"ros{gt}", name=f"ros{gt}")
                nc.vector.tensor_copy(out=ro_s[:gsz, :], in_=ro_ps[gt][:])
                nc.sync.dma_start(out=xein.ap()[gt * P:gt * P + gsz, :], in_=ro_s[:gsz, :])
            nc.gpsimd.collective_compute(
                "AllReduce", ALU.add, replica_groups=RG,
                ins=[xein.ap()], outs=[xeout.ap()],
            )
            if dbg:
                nc.sync.dma_start(out=dbgxe.ap(), in_=xeout.ap())
            xeT = pp5.tile([16, G], f32, tag="xeT")
            for gt in range(GT):
                gsz = min(P, G - gt * P)
                xa = pp5.tile([P, 17], f32, tag=f"xa{gt}", name=f"xa{gt}")
                nc.sync.dma_start(out=xa[:gsz, :], in_=xeout.ap()[gt * P:gt * P + gsz, :])
                cm2 = pp5.tile([P, 1], f32, tag=f"cm2{gt}", name=f"cm2{gt}")
                nc.vector.tensor_scalar_max(out=cm2[:gsz], in0=xa[:gsz, 16:17], scalar1=1.0)
                inv2 = pp5.tile([P, 1], f32, tag=f"inv2{gt}", name=f"inv2{gt}")
                nc.vector.reciprocal(out=inv2[:gsz], in_=cm2[:gsz])
                xe = pp5.tile([P, 16], f32, tag=f"xe{gt}", name=f"xe{gt}")
                nc.vector.tensor_scalar(
                    out=xe[:gsz], in0=xa[:gsz, 0:16], scalar1=inv2[:gsz],
                    scalar2=None, op0=ALU.mult,
                )
                tp = ps5.tile([16, P], f32, tag=f"tp{gt}", name=f"tp{gt}")
                nc.tensor.transpose(out=tp[:, :gsz], in_=xe[:gsz, :], identity=ident_s[:gsz, :gsz])
                nc.vector.tensor_copy(out=xeT[:, gt * P:gt * P + gsz], in_=tp[:, :gsz])

            def bn_t(src_ap, Fd, gl, bl, dest):
                s = pp5.tile([Fd, 1], f32, tag=f"bns{Fd}", name=f"bns{Fd}")
                nc.vector.tensor_reduce(out=s[:], in_=src_ap, axis=mybir.AxisListType.X, op=ALU.add)
                mu5 = pp5.tile([Fd, 1], f32, tag=f"bnmu{Fd}", name=f"bnmu{Fd}")
                nc.vector.tensor_scalar(
                    out=mu5[:], in0=s[:], scalar1=1.0 / G, scalar2=None, op0=ALU.mult
                )
                d = pp5.tile([Fd, G], f32, tag=f"bnd{Fd}", name=f"bnd{Fd}")
                nc.vector.tensor_scalar(
                    out=d[:], in0=src_ap, scalar1=mu5[:], scalar2=None, op0=ALU.subtract
                )
                sq5 = pp5.tile([Fd, G], f32, tag=f"bnsq{Fd}", name=f"bnsq{Fd}")
                nc.vector.tensor_tensor(out=sq5[:], in0=d[:], in1=d[:], op=ALU.mult)
                v = pp5.tile([Fd, 1], f32, tag=f"bnv{Fd}", name=f"bnv{Fd}")
                nc.vector.tensor_reduce(out=v[:], in_=sq5[:], axis=mybir.AxisListType.X, op=ALU.add)
                nc.vector.tensor_scalar(
                    out=v[:], in0=v[:], scalar1=1.0 / G, scalar2=EPS, op0=ALU.mult, op1=ALU.add
                )
                sd5 = pp5.tile([Fd, 1], f32, tag=f"bnsd{Fd}", name=f"bnsd{Fd}")
                nc.scalar.sqrt(out=sd5[:], in_=v[:])
                rs5 = pp5.tile([Fd, 1], f32, tag=f"bnrs{Fd}", name=f"bnrs{Fd}")
                nc.vector.reciprocal(out=rs5[:], in_=sd5[:])
                sc5 = pp5.tile([Fd, 1], f32, tag=f"bnsc{Fd}", name=f"bnsc{Fd}")
                nc.vector.tensor_tensor(out=sc5[:], in0=gl, in1=rs5[:], op=ALU.mult)
                nc.vector.tensor_scalar(
                    out=dest, in0=d[:], scalar1=sc5[:], scalar2=bl, op0=ALU.mult, op1=ALU.add
                )

            bn1 = pp5.tile([16, G], f32, tag="bn1")
            bn_t(xeT[:], 16, cols_s["gl1"][:], cols_s["bl1"][:], bn1[:])
            z1p = ps5.tile([16, G], f32, tag="z1p")
            nc.tensor.matmul(out=z1p[:], lhsT=Ws_s["W1"][:], rhs=bn1[:], start=True, stop=True)
            zs1 = pp5.tile([16, G], f32, tag="zs1")
            nc.scalar.activation(out=zs1[:], in_=z1p[:], func=AF.Relu, bias=cols_s["bW1"][:], scale=1.0)
            bn2a = pp5.tile([16, G], f32, tag="bn2a")
            bn_t(zs1[:], 16, cols_s["gl2a"][:], cols_s["bl2a"][:], bn2a[:])
            bn2b = pp5.tile([16, G], f32, tag="bn2b")
            bn_t(xeT[:], 16, cols_s["gl2b"][:], cols_s["bl2b"][:], bn2b[:])
            z2p = ps5.tile([16, G], f32, tag="z2p")
            nc.tensor.matmul(out=z2p[:], lhsT=Ws_s["W2a"][:], rhs=bn2a[:], start=True, stop=False)
            nc.tensor.matmul(out=z2p[:], lhsT=Ws_s["W2b"][:], rhs=bn2b[:], start=False, stop=True)
            zs2 = pp5.tile([16, G], f32, tag="zs2")
            nc.scalar.activation(out=zs2[:], in_=z2p[:], func=AF.Relu, bias=cols_s["bW2"][:], scale=1.0)
            bn3a = pp5.tile([16, G], f32, tag="bn3a")
            bn_t(zs2[:], 16, cols_s["gl3a"][:], cols_s["bl3a"][:], bn3a[:])
            bn3b = pp5.tile([16, G], f32, tag="bn3b")
            bn_t(xeT[:], 16, cols_s["gl3b"][:], cols_s["bl3b"][:], bn3b[:])
            z3p = ps5.tile([16, G], f32, tag="z3p")
            nc.tensor.matmul(out=z3p[:], lhsT=Ws_s["W3a"][:], rhs=bn3a[:], start=True, stop=False)
            nc.tensor.matmul(out=z3p[:], lhsT=Ws_s["W3b"][:], rhs=bn3b[:], start=False, stop=True)
            z3 = pp5.tile([16, G], f32, tag="z3")
            nc.scalar.activation(out=z3[:], in_=z3p[:], func=AF.Relu, bias=cols_s["bW3"][:], scale=1.0)
            ofp = ps5.tile([1, G], f32, tag="ofp")
            nc.tensor.matmul(out=ofp[:], lhsT=Ws_s["Wf"][:], rhs=z3[:], start=True, stop=True)
            ofs = pp5.tile([1, G], f32, tag="ofs")
            nc.vector.tensor_scalar(
                out=ofs[:], in0=ofp[:], scalar1=cols_s["bWf"][:], scalar2=None, op0=ALU.add
            )
            nc.sync.dma_start(out=out_t.ap(), in_=ofs[:])

    nc.compile()
    return nc


def run(inputs, n_cores=8, G=256, cfg_overrides=None, trace=False, enable_asserts=False):
    from concourse.bass_utils import run_bass_kernel_spmd

    in_maps, cfg = build_host_data(
        inputs["x"], inputs["edge_index"], inputs["batch"], n_cores, G
    )
    if cfg_overrides:
        cfg.update(cfg_overrides)
    add_weights(in_maps, inputs)
    nc = build_program(cfg, enable_asserts=enable_asserts)
    res = run_bass_kernel_spmd(nc, in_maps, core_ids=list(range(n_cores)), trace=trace)
    out = res.results[0]["out"].reshape(G, 1)
    return out, res, cfg


def kernel(**inputs):
    """Full inputs -> full [256, 1] output. Shards internally across 8 cores."""
    out, _, _ = run(inputs, n_cores=8, G=256)
    return np.asarray(out, np.float32)


# revision 13
# speedup vs baseline: 4.0834x; 1.0706x over previous
"""Trainium2 Bass kernel for nn_D2RLCritic (gnn_message_passing).

Self-contained: kernel(**inputs) takes the FULL unsharded inputs (as from
setup_inputs()) and returns the FULL [256, 1] output, running an SPMD Bass
kernel across 8 NeuronCores.

Design: dst-sharded graph (12544 nodes/core, 98 blocks of 128). Per-edge
feature fetches use gpsimd ap_gather from SBUF-resident transposed feature
stripes ([128, 12544]: partition 16r+f = feature f of node range r), with
per-16-partition-group index streams. L1 projects x@w1l per node first, so
both layers gather 16-dim rows. Gathered columns are transposed on PE into
slot-major E tiles; a bf16 one-hot (dst within block) matmul accumulates the
segment sum in PSUM. Degrees/masks are host-precomputed index tables.
"""

import numpy as np
from contextlib import ExitStack

from concourse import bass, bacc, mybir, tile
from concourse.mybir import AluOpType as ALU
from concourse.mybir import ActivationFunctionType as AF

P = 128
NR = 8
dt = mybir.dt
EPS = 1e-5
CW = 4096


def build_host_data(x, edge_index, batch, n_cores, G):
    assert n_cores == NR
    x = np.ascontiguousarray(np.asarray(x, np.float32))
    src_g = np.asarray(edge_index[0], np.int64)
    dst_g = np.asarray(edge_index[1], np.int64)
    batch = np.asarray(batch, np.int64)
    N0, F = x.shape
    NS = ((N0 + NR * P - 1) // (NR * P)) * P  # 12544
    Npad = NS * NR
    NB = NS // P

    xp = np.zeros((Npad, F), np.float32)
    xp[:N0] = x
    deg = np.bincount(dst_g, minlength=Npad).astype(np.int64)
    batchp = np.full(Npad, -1, np.int64)
    batchp[:N0] = batch

    # per-core edge sort and cell counts
    per_s, per_d, per_cnt = [], [], []
    for k in range(NR):
        m = (dst_g >= k * NS) & (dst_g < (k + 1) * NS)
        s = src_g[m]
        d = dst_g[m] - k * NS
        blk = d >> 7
        rng = s // NS
        order = np.lexsort((s, rng, blk))
        s, d, blk, rng = s[order], d[order], blk[order], rng[order]
        cnt = np.zeros((NB, NR), np.int64)
        np.add.at(cnt, (blk, rng), 1)
        per_s.append(s)
        per_d.append(d)
        per_cnt.append(cnt)
    cnts = np.stack(per_cnt)            # [NR_cores, NB, NR]
    W = cnts.max(axis=0)                # [NB, NR]

    S = np.zeros((NB, NR), np.int64)    # stream offset of cell (b, r)
    S[1:] = np.cumsum(W, axis=0)[:-1]
    L_r = S[-1] + W[-1]
    L = int(((L_r.max() + P - 1) // P) * P)
    NCH = (L + CW - 1) // CW

    # pieces: per block, ordered list of (r, window, drel_col); windows are
    # 128-col spans of all 8 streams (one transposed square serves 8 ranges)
    pieces = []
    npiece = 0
    for b in range(NB):
        plist = []
        for r in range(NR):
            a, z = int(S[b, r]), int(S[b, r] + W[b, r])
            for win in range(a // P, (z + P - 1) // P):
                plist.append((r, win, npiece))
                npiece += 1
        plist.sort(key=lambda t: (t[1], t[0]))
        pieces.append(plist)
    NPIECE = npiece
    in_maps = []
    for k in range(NR):
        s, d, cnt = per_s[k], per_d[k], per_cnt[k]
        # cell start offsets in the sorted edge array
        estart = np.concatenate([[0], np.cumsum(cnt.ravel())])[:-1].reshape(NB, NR)
        # gather index streams, wrapped per 16-partition group
        apg = np.zeros((P, L // 16), np.int16)
        dstrel = np.full((NR, L), -1, np.int64)
        for r in range(NR):
            stream = np.zeros(L, np.int64)
            for b in range(NB):
                n_e = int(cnt[b, r])
                if n_e:
                    e0 = estart[b, r]
                    stream[S[b, r]: S[b, r] + n_e] = s[e0: e0 + n_e] - r * NS
                    dstrel[r, S[b, r]: S[b, r] + n_e] = d[e0: e0 + n_e] - b * P
            apg[16 * r: 16 * (r + 1), :] = (
                stream.reshape(L // 16, 16).T.astype(np.int16)
            )
        # mtbig: [128, NPIECE*128] bf16 inverse-degree-scaled one-hots:
        # mt[p, pc*128+dd] = 1/deg(dst) if slot p of piece pc maps to block
        # offset dd, else 0
        import ml_dtypes
        drel = np.full((P, NPIECE), -1, np.int64)
        pblk = np.zeros(NPIECE, np.int64)
        for b in range(NB):
            for (r, win, pc) in pieces[b]:
                a, z = int(S[b, r]), int(S[b, r] + W[b, r])
                lo, hi = win * P, (win + 1) * P
                aa, zz = max(a, lo), min(z, hi)
                col = np.full(P, -1, np.int64)
                col[aa - lo: zz - lo] = dstrel[r, aa: zz]
                drel[:, pc] = col
                pblk[pc] = b
        onehot = (drel[:, :, None] == np.arange(P)[None, None, :])
        nodes_all = np.arange(NS) + k * NS
        invd_full = (1.0 / np.maximum(deg[nodes_all], 1)).astype(np.float32)
        inv_pc = invd_full.reshape(NB, P)[pblk]          # [NPIECE, 128]
        mtbig = (onehot * inv_pc[None, :, :]).astype(ml_dtypes.bfloat16)
        mtbig = mtbig.reshape(P, NPIECE * P)
        nodes = np.arange(NS) + k * NS
        grel = np.where(nodes < N0, batchp[nodes], -1).astype(np.float32)
        in_maps.append(dict(
            xown=xp[k * NS:(k + 1) * NS],
            apgidx=apg,
            mtbig=mtbig,
            grel=grel.reshape(NB, P).T.copy(),
        ))

    cfg = dict(N=N0, NS=NS, NB=NB, F=F, G=G, NPIECE=NPIECE, L=L, NCH=NCH,
               pieces=pieces, n_cores=NR)
    return in_maps, cfg


def add_weights(in_maps, inputs):
    f32 = np.float32
    w = {}
    w["w1cat"] = np.concatenate(
        [np.asarray(inputs["w1l"], f32), np.asarray(inputs["w1r"], f32)], axis=1
    )  # [64, 32]
    w["w2l"] = np.asarray(inputs["w2l"], f32)
    w["w2r"] = np.asarray(inputs["w2r"], f32)
    for name in ("b1l", "b2l", "g1", "be1"):
        w[name] = np.asarray(inputs[name], f32).reshape(16, 1)
    for name in ("gl1", "bl1", "bW1", "bW2", "bW3"):
        w[name] = np.asarray(inputs[name], f32).reshape(16, 1)
    w["bWf"] = np.asarray(inputs["bWf"], f32).reshape(1, 1)
    for name in ("gl2", "bl2", "gl3", "bl3"):
        v = np.asarray(inputs[name], f32).reshape(32, 1)
        w[name + "a"], w[name + "b"] = v[:16].copy(), v[16:].copy()
    w["W1"] = np.asarray(inputs["W1"], f32)
    w["Wf"] = np.asarray(inputs["Wf"], f32)
    for name in ("W2", "W3"):
        v = np.asarray(inputs[name], f32)
        w[name + "a"], w[name + "b"] = v[:16].copy(), v[16:].copy()
    for m in in_maps:
        m.update(w)
    return in_maps


def build_program(cfg, enable_asserts=False):
    NCORES = cfg["n_cores"]
    N, NS, NB, F, G = cfg["N"], cfg["NS"], cfg["NB"], cfg["F"], cfg["G"]
    NPIECE, L, NCH = cfg["NPIECE"], cfg["L"], cfg["NCH"]
    pieces = cfg["pieces"]
    GT = (G + P - 1) // P
    f32, bf16 = dt.float32, dt.bfloat16

    nc = bacc.Bacc(
        "TRN2", target_bir_lowering=False, debug=False,
        enable_asserts=enable_asserts, num_devices=NCORES,
    )
    RG = [list(range(NCORES))]

    xown_in = nc.dram_tensor("xown", [NS, F], f32, kind="ExternalInput")
    apg_in = nc.dram_tensor("apgidx", [P, L // 16], dt.int16, kind="ExternalInput")
    mtbig_in = nc.dram_tensor("mtbig", [P, NPIECE * P], dt.bfloat16, kind="ExternalInput")
    grel_in = nc.dram_tensor("grel", [P, NB], f32, kind="ExternalInput")
    w1cat_in = nc.dram_tensor("w1cat", [F, 32], f32, kind="ExternalInput")
    w2l_in = nc.dram_tensor("w2l", [16, 16], f32, kind="ExternalInput")
    w2r_in = nc.dram_tensor("w2r", [16, 16], f32, kind="ExternalInput")
    row_ins = {
        name: nc.dram_tensor(name, [16, 1], f32, kind="ExternalInput")
        for name in ("b1l", "b2l", "g1", "be1")
    }
    col_names = ("gl1", "bl1", "bW1", "gl2a", "gl2b", "bl2a", "bl2b",
                 "gl3a", "gl3b", "bl3a", "bl3b", "bW2", "bW3")
    col_ins = {
        name: nc.dram_tensor(name, [16, 1], f32, kind="ExternalInput")
        for name in col_names
    }
    col_ins["bWf"] = nc.dram_tensor("bWf", [1, 1], f32, kind="ExternalInput")
    W_ins = {
        name: nc.dram_tensor(name, [16, shp1], f32, kind="ExternalInput")
        for name, shp1 in (
            ("W1", 16), ("W2a", 16), ("W2b", 16), ("W3a", 16), ("W3b", 16), ("Wf", 1),
        )
    }
    out_t = nc.dram_tensor("out", [1, G], f32, kind="ExternalOutput")
    dbg = cfg.get("debug")
    if dbg:
        dbgy = nc.dram_tensor("dbgy", [P, NS], f32, kind="ExternalOutput")
        dbgh1 = nc.dram_tensor("dbgh1", [P, NS], f32, kind="ExternalOutput")
        dbgxe = nc.dram_tensor("dbgxe", [G, 17], f32, kind="ExternalOutput")


    y1tsh = nc.dram_tensor("y1tsh", [16, NS], f32, kind="Internal")
    y1tall = nc.dram_tensor("y1tall", [P, NS], f32, kind="Internal", addr_space="Shared")
    h1tsh = nc.dram_tensor("h1tsh", [16, NS], f32, kind="Internal")
    h1tall = nc.dram_tensor("h1tall", [P, NS], f32, kind="Internal", addr_space="Shared")
    stin = nc.dram_tensor("stin", [16, 2], f32, kind="Internal")
    stout = nc.dram_tensor("stout", [16, 2], f32, kind="Internal", addr_space="Shared")
    xein = nc.dram_tensor("xein", [G, 17], f32, kind="Internal")
    xeout = nc.dram_tensor("xeout", [G, 17], f32, kind="Internal", addr_space="Shared")

    iota128_t = nc.inline_tensor(
        np.broadcast_to(np.arange(P, dtype=np.float32), (P, P)).copy(), "iota128"
    )
    iotag_t = nc.inline_tensor(
        np.broadcast_to(np.arange(G, dtype=np.float32), (P, G)).copy(), "iotag"
    )
    ident_t = nc.inline_tensor(np.eye(P, dtype=np.float32), "ident")

    with tile.TileContext(nc) as tc, ExitStack() as top:
        persist = top.enter_context(tc.tile_pool(name="persist", bufs=1))

        iota_f = persist.tile([P, P], f32)
        nc.sync.dma_start(out=iota_f[:], in_=iota128_t.ap())
        iotag_s = persist.tile([P, G], f32)
        nc.sync.dma_start(out=iotag_s[:], in_=iotag_t.ap())
        ident_s = persist.tile([P, P], f32)
        nc.sync.dma_start(out=ident_s[:], in_=ident_t.ap())
        apg_s = persist.tile([P, L // 16], dt.int16)
        nc.sync.dma_start(out=apg_s[:], in_=apg_in.ap())
        grel_s = persist.tile([P, NB], f32)
        nc.sync.dma_start(out=grel_s[:], in_=grel_in.ap())
        w1cat_s = persist.tile([F, 32], f32)
        nc.sync.dma_start(out=w1cat_s[:], in_=w1cat_in.ap())
        w2l_s = persist.tile([16, 16], f32)
        nc.sync.dma_start(out=w2l_s[:], in_=w2l_in.ap())
        w2r_s = persist.tile([16, 16], f32)
        nc.sync.dma_start(out=w2r_s[:], in_=w2r_in.ap())
        rows_s = {}
        for name, t in row_ins.items():
            rows_s[name] = persist.tile([16, 1], f32, tag=f"row_{name}", name=f"row_{name}")
            nc.sync.dma_start(out=rows_s[name][:], in_=t.ap())
        cols_s = {}
        for name, t in col_ins.items():
            cols_s[name] = persist.tile(list(t.shape), f32, tag=f"col_{name}", name=f"col_{name}")
            nc.sync.dma_start(out=cols_s[name][:], in_=t.ap())
        Ws_s = {}
        for name, t in W_ins.items():
            Ws_s[name] = persist.tile(list(t.shape), f32, tag=f"W_{name}", name=f"W_{name}")
            nc.sync.dma_start(out=Ws_s[name][:], in_=t.ap())

        w1cat_b = persist.tile([F, 32], bf16)
        nc.vector.tensor_copy(out=w1cat_b[:], in_=w1cat_s[:])
        w2l_b = persist.tile([16, 16], bf16)
        nc.vector.tensor_copy(out=w2l_b[:], in_=w2l_s[:])
        w2r_b = persist.tile([16, 16], bf16)
        nc.vector.tensor_copy(out=w2r_b[:], in_=w2r_s[:])
        ident16_b = persist.tile([16, 16], bf16)
        nc.vector.tensor_copy(out=ident16_b[:], in_=ident_s[:16, :16])

        stripe_s = persist.tile([P, NS], f32)        # y1T then h1T (gathered)
        ytown = persist.tile([16, NS], f32)          # y1T own, then h1T own
        xrbt_own = persist.tile([16, NS], bf16)      # (x @ w1r + b1l)^T own
        statacc = persist.tile([16, 2], f32)
        nc.vector.memset(statacc[:], 0.0)
        a_col = persist.tile([16, 1], f32, tag="a_col")
        c_col = persist.tile([16, 1], f32, tag="c_col")

        # warmup: load ap_gather ucode early
        with tc.tile_pool(name="warm", bufs=1) as wp:
            wi = wp.tile([P, 16], dt.int16)
            nc.gpsimd.memset(wi[:], 0)
            wo = wp.tile([P, 16], f32)
            nc.gpsimd.ap_gather(
                out_ap=wo[:], in_ap=iota_f[:], idxs_ap=wi[:, 0:1],
                channels=P, num_elems=P, d=1, num_idxs=16,
            )

        # ================= L1 prep: y1T own + xrbT =================
        with tc.tile_pool(name="p1", bufs=3) as pl, tc.tile_pool(
            name="p1ps", bufs=2, space="PSUM"
        ) as ps1, tc.tile_pool(name="p1s", bufs=3) as sb1:
            for b in range(NB):
                xb = pl.tile([P, F], f32, tag="xb")
                nc.sync.dma_start(out=xb[:], in_=xown_in.ap()[b * P:(b + 1) * P, :])
                xTp = ps1.tile([F, P], f32, tag="xTp", name="xTp")
                nc.tensor.transpose(out=xTp[:], in_=xb[:], identity=ident_s[:])
                xT_s = sb1.tile([F, P], bf16, tag="xTs")
                nc.vector.tensor_copy(out=xT_s[:], in_=xTp[:])
                y1p = ps1.tile([16, P], f32, tag="y1p", name="y1p")
                nc.tensor.matmul(out=y1p[:], lhsT=w1cat_b[:, 0:16], rhs=xT_s[:],
                                 start=True, stop=True)
                nc.vector.tensor_copy(out=ytown[:, b * P:(b + 1) * P], in_=y1p[:])
                xrp = ps1.tile([16, P], f32, tag="xrp", name="xrp")
                nc.tensor.matmul(out=xrp[:], lhsT=w1cat_b[:, 16:32], rhs=xT_s[:],
                                 start=True, stop=True)
                nc.vector.tensor_scalar(
                    out=xrbt_own[:, b * P:(b + 1) * P], in0=xrp[:],
                    scalar1=rows_s["b1l"][:], scalar2=None, op0=ALU.add,
                )
        nc.sync.dma_start(out=y1tsh.ap(), in_=ytown[:])
        nc.gpsimd.collective_compute(
            "AllGather", ALU.bypass, replica_groups=RG,
            ins=[y1tsh.ap()], outs=[y1tall.ap()],
        )
        nc.sync.dma_start(out=stripe_s[:], in_=y1tall.ap())
        if dbg:
            nc.sync.dma_start(out=dbgy.ap(), in_=y1tall.ap())

        # ================= shared edge-layer emitter =================
        WPC = CW // P   # windows per chunk
        MTB = 16        # one-hot pieces per DMA batch
        fence_pool = top.enter_context(tc.tile_pool(name="fence", bufs=2))

        def pool_fence():
            """Order later gpsimd work after the stripe/idx loads: gpsimd is
            in-order, and this op's reads are dependency-tracked."""
            fp = fence_pool.tile([1, 4], f32, tag="fence", name="fence")
            nc.gpsimd.tensor_tensor(out=fp[:], in0=stripe_s[0:1, 0:4],
                                    in1=apg_s[0:1, 0:8].bitcast(f32),
                                    op=ALU.add)

        def emit_layer(layer, epilogue, start_extra):
            """Gather + window transposes + psdT accumulation per block.

            psdT[f, d] = sum over pieces of E_piece^T @ MTinv_piece; the
            host-prepared MTinv folds 1/deg. start_extra(b, psd) may emit an
            initial accumulating matmul (returns True if it started the
            group)."""
            pool_fence()
            with tc.tile_pool(name=f"ch{layer}", bufs=3) as chp, tc.tile_pool(
                name=f"sqps{layer}", bufs=2, space="PSUM"
            ) as sqps, tc.tile_pool(name=f"sq{layer}", bufs=8) as sqp, tc.tile_pool(
                name=f"mt{layer}", bufs=3
            ) as mtp, tc.tile_pool(name=f"psd{layer}", bufs=2, space="PSUM") as psdp, \
                 tc.tile_pool(name=f"ep{layer}", bufs=3) as epp, tc.tile_pool(
                name=f"epps{layer}", bufs=1, space="PSUM"
            ) as epps:
                chunks = {}
                squares = {}
                mtts = {}
                next_ch = 0

                def ensure_window(win):
                    nonlocal next_ch
                    if win in squares:
                        return
                    while next_ch <= win // WPC and next_ch < NCH:
                        cw = min(CW, L - next_ch * CW)
                        ct = chp.tile([P, CW], f32, tag="chunk")
                        nc.gpsimd.ap_gather(
                            out_ap=ct[:, 0:cw], in_ap=stripe_s[:],
                            idxs_ap=apg_s[:, next_ch * (CW // 16):
                                          next_ch * (CW // 16) + cw // 16],
                            channels=P, num_elems=NS, d=1, num_idxs=cw,
                        )
                        chunks[next_ch] = ct
                        next_ch += 1
                    cc = (win % WPC) * P
                    sq_ps = sqps.tile([P, P], f32, tag="sqps", name="sqps")
                    nc.tensor.transpose(
                        out=sq_ps[:], in_=chunks[win // WPC][:, cc:cc + P],
                        identity=ident_s[:],
                    )
                    sq = sqp.tile([P, P], bf16, tag="sq")
                    nc.scalar.activation(out=sq[:], in_=sq_ps[:], func=AF.Copy)
                    squares[win] = sq

                def ensure_mt(pc):
                    g = pc // MTB
                    if g not in mtts:
                        g0 = g * MTB * P
                        gw = min(MTB * P, NPIECE * P - g0)
                        mt = mtp.tile([P, MTB * P], bf16, tag="mtb")
                        nc.sync.dma_start(out=mt[:, 0:gw],
                                          in_=mtbig_in.ap()[:, g0:g0 + gw])
                        mtts[g] = mt
                    return mtts[g], (pc % MTB) * P

                for b in range(NB):
                    for (r, win, pc) in pieces[b]:
                        ensure_window(win)
                        ensure_mt(pc)
                    psd = psdp.tile([16, P], f32, tag="psd", name="psd")
                    started = start_extra(b, psd)
                    np_b = len(pieces[b])
                    for i, (r, win, pc) in enumerate(pieces[b]):
                        mt, mo = ensure_mt(pc)
                        nc.tensor.matmul(
                            out=psd[:], lhsT=squares[win][:, 16 * r: 16 * (r + 1)],
                            rhs=mt[:, mo: mo + P],
                            start=(i == 0 and not started), stop=(i == np_b - 1),
                            skip_group_check=True,
                        )
                    epilogue(b, psd, epp, epps)

        # ================= L1 main =================
        def l1_start(b, psd):
            nc.tensor.matmul(
                out=psd[:], lhsT=ident16_b[:],
                rhs=xrbt_own[:, b * P:(b + 1) * P],
                start=True, stop=False, skip_group_check=True,
            )
            return True

        def l1_epilogue(b, psd, epp, epps):
            h1t = epp.tile([16, P], f32, tag="h1t", name="h1t")
            nc.scalar.activation(out=h1t[:], in_=psd[:], func=AF.Relu)
            nc.vector.tensor_copy(out=ytown[:, b * P:(b + 1) * P], in_=h1t[:])
            sq1 = epp.tile([16, P], f32, tag="sq1", name="sq1")
            nc.scalar.square(out=sq1[:], in_=h1t[:])
            red = epp.tile([16, 2], f32, tag="red", name="red")
            nc.vector.tensor_reduce(out=red[:, 0:1], in_=h1t[:],
                                    axis=mybir.AxisListType.X, op=ALU.add)
            nc.vector.tensor_reduce(out=red[:, 1:2], in_=sq1[:],
                                    axis=mybir.AxisListType.X, op=ALU.add)
            nc.vector.tensor_tensor(out=statacc[:], in0=statacc[:], in1=red[:],
                                    op=ALU.add)

        emit_layer(1, l1_epilogue, l1_start)

        nc.sync.dma_start(out=h1tsh.ap(), in_=ytown[:])
        nc.gpsimd.collective_compute(
            "AllGather", ALU.bypass, replica_groups=RG,
            ins=[h1tsh.ap()], outs=[h1tall.ap()],
        )
        with tc.tile_pool(name="st", bufs=1) as pst:
            sts = pst.tile([16, 2], f32)
            nc.vector.tensor_copy(out=sts[:], in_=statacc[:])
            nc.sync.dma_start(out=stin.ap(), in_=sts[:])
        nc.gpsimd.collective_compute(
            "AllReduce", ALU.add, replica_groups=RG,
            ins=[stin.ap()], outs=[stout.ap()],
        )
        nc.sync.dma_start(out=stripe_s[:], in_=h1tall.ap())
        if dbg:
            nc.sync.dma_start(out=dbgh1.ap(), in_=h1tall.ap())

        # ---- BN affine from stats (pad nodes contribute relu(b1l) each) ----
        NPAD = NS * NCORES - N
        with tc.tile_pool(name="ph3", bufs=1) as pp3:
            st = pp3.tile([16, 2], f32)
            nc.sync.dma_start(out=st[:], in_=stout.ap())
            rb = pp3.tile([16, 2], f32, tag="rb")
            nc.scalar.activation(out=rb[:, 0:1], in_=rows_s["b1l"][:], func=AF.Relu)
            nc.scalar.square(out=rb[:, 1:2], in_=rb[:, 0:1])
            nc.vector.tensor_scalar(
                out=rb[:], in0=rb[:], scalar1=-float(NPAD), scalar2=None, op0=ALU.mult
            )
            nc.vector.tensor_tensor(out=st[:], in0=st[:], in1=rb[:], op=ALU.add)
            mu = pp3.tile([16, 1], f32, tag="mu")
            nc.vector.tensor_scalar(
                out=mu[:], in0=st[:, 0:1], scalar1=1.0 / N, scalar2=None, op0=ALU.mult
            )
            var = pp3.tile([16, 1], f32, tag="var")
            nc.vector.tensor_scalar(
                out=var[:], in0=st[:, 1:2], scalar1=1.0 / N, scalar2=None, op0=ALU.mult
            )
            musq = pp3.tile([16, 1], f32, tag="musq")
            nc.vector.tensor_tensor(out=musq[:], in0=mu[:], in1=mu[:], op=ALU.mult)
            nc.vector.tensor_tensor(out=var[:], in0=var[:], in1=musq[:], op=ALU.subtract)
            nc.vector.tensor_scalar(
                out=var[:], in0=var[:], scalar1=EPS, scalar2=None, op0=ALU.add
            )
            sd = pp3.tile([16, 1], f32, tag="sd")
            nc.scalar.sqrt(out=sd[:], in_=var[:])
            rstd = pp3.tile([16, 1], f32, tag="rstd")
            nc.vector.reciprocal(out=rstd[:], in_=sd[:])
            nc.vector.tensor_tensor(out=a_col[:], in0=rows_s["g1"][:], in1=rstd[:], op=ALU.mult)
            cc1 = pp3.tile([16, 1], f32, tag="cc1")
            nc.vector.tensor_tensor(out=cc1[:], in0=a_col[:], in1=mu[:], op=ALU.mult)
            nc.vector.tensor_tensor(out=c_col[:], in0=rows_s["be1"][:], in1=cc1[:], op=ALU.subtract)

        # ================= L2 main =================
        ro_pool = top.enter_context(tc.tile_pool(name="rops", bufs=1, space="PSUM"))
        ro_ps = [
            ro_pool.tile([min(P, G - gt * P), 17], f32, tag=f"ro{gt}", name=f"ro{gt}")
            for gt in range(GT)
        ]

        def l2_start(b, psd):
            return False

        def l2_epilogue(b, psd, epp, epps):
            m2 = epp.tile([16, P], bf16, tag="m2", name="m2")
            nc.vector.tensor_scalar(
                out=m2[:], in0=psd[:], scalar1=a_col[:], scalar2=c_col[:],
                op0=ALU.mult, op1=ALU.add,
            )
            bnh1 = epp.tile([16, P], bf16, tag="bnh1", name="bnh1")
            nc.vector.tensor_scalar(
                out=bnh1[:], in0=ytown[:, b * P:(b + 1) * P],
                scalar1=a_col[:], scalar2=c_col[:], op0=ALU.mult, op1=ALU.add,
            )
            h2p = epps.tile([16, P], f32, tag="h2p", name="h2p")
            nc.tensor.matmul(out=h2p[:], lhsT=w2l_b[:], rhs=m2[:], start=True, stop=False,
                             skip_group_check=True)
            nc.tensor.matmul(out=h2p[:], lhsT=w2r_b[:], rhs=bnh1[:], start=False, stop=True,
                             skip_group_check=True)
            h2t = epp.tile([16, P], f32, tag="h2t", name="h2t")
            nc.scalar.activation(out=h2t[:], in_=h2p[:], func=AF.Relu,
                                 bias=rows_s["b2l"][:], scale=1.0)
            h2ep = epps.tile([P, 16], f32, tag="h2ep", name="h2ep")
            nc.tensor.transpose(out=h2ep[:], in_=h2t[:], identity=ident_s[:16, :16])
            h2e = epp.tile([P, 17], f32, tag="h2e", name="h2e")
            nc.vector.tensor_copy(out=h2e[:, 0:16], in_=h2ep[:])
            nc.vector.memset(h2e[:, 16:17], 1.0)
            MTg = epp.tile([P, G], f32, tag="MTg", name="MTg")
            nc.vector.tensor_scalar(
                out=MTg[:], in0=iotag_s[:], scalar1=grel_s[:, b:b + 1],
                scalar2=None, op0=ALU.is_equal,
            )
            for gt in range(GT):
                gsz = min(P, G - gt * P)
                nc.tensor.matmul(
                    out=ro_ps[gt][:], lhsT=MTg[:, gt * P:gt * P + gsz],
                    rhs=h2e[:], start=(b == 0), stop=(b == NB - 1),
                    skip_group_check=True,
                )

        emit_layer(2, l2_epilogue, l2_start)

        # ================= readout =================
        with tc.tile_pool(name="ph5", bufs=1) as pp5, tc.tile_pool(
            name="ph5ps", bufs=1, space="PSUM"
        ) as ps5:
            for gt in range(GT):
                gsz = min(P, G - gt * P)
                ro_s = pp5.tile([P, 17], f32, tag=f"ros{gt}", name=f"ros{gt}")
                nc.vector.tensor_copy(out=ro_s[:gsz, :], in_=ro_ps[gt][:])
                nc.sync.dma_start(out=xein.ap()[gt * P:gt * P + gsz, :], in_=ro_s[:gsz, :])
            nc.gpsimd.collective_compute(
                "AllReduce", ALU.add, replica_groups=RG,
                ins=[xein.ap()], outs=[xeout.ap()],
            )
            if dbg:
                nc.sync.dma_start(out=dbgxe.ap(), in_=xeout.ap())
            xeT = pp5.tile([16, G], f32, tag="xeT")
            for gt in range(GT):
                gsz = min(P, G - gt * P)
                xa = pp5.tile([P, 17], f32, tag=f"xa{gt}", name=f"xa{gt}")
                nc.sync.dma_start(out=xa[:gsz, :], in_=xeout.ap()[gt * P:gt * P + gsz, :])
                cm2 = pp5.tile([P, 1], f32, tag=f"cm2{gt}", name=f"cm2{gt}")
                nc.vector.tensor_scalar_max(out=cm2[:gsz], in0=xa[:gsz, 16:17], scalar1=1.0)
                inv2 = pp5.tile([P, 1], f32, tag=f"inv2{gt}", name=f"inv2{gt}")
                nc.vector.reciprocal(out=inv2[:gsz], in_=cm2[:gsz])
                xe = pp5.tile([P, 16], f32, tag=f"xe{gt}", name=f"xe{gt}")
                nc.vector.tensor_scalar(
                    out=xe[:gsz], in0=xa[:gsz, 0:16], scalar1=inv2[:gsz],
                    scalar2=None, op0=ALU.mult,
                )
                tp = ps5.tile([16, P], f32, tag=f"tp{gt}", name=f"tp{gt}")
                nc.tensor.transpose(out=tp[:, :gsz], in_=xe[:gsz, :], identity=ident_s[:gsz, :gsz])
                nc.vector.tensor_copy(out=xeT[:, gt * P:gt * P + gsz], in_=tp[:, :gsz])

            def bn_t(src_ap, Fd, gl, bl, dest):
                s = pp5.tile([Fd, 1], f32, tag=f"bns{Fd}", name=f"bns{Fd}")
                nc.vector.tensor_reduce(out=s[:], in_=src_ap, axis=mybir.AxisListType.X, op=ALU.add)
                mu5 = pp5.tile([Fd, 1], f32, tag=f"bnmu{Fd}", name=f"bnmu{Fd}")
                nc.vector.tensor_scalar(
                    out=mu5[:], in0=s[:], scalar1=1.0 / G, scalar2=None, op0=ALU.mult
                )
                d = pp5.tile([Fd, G], f32, tag=f"bnd{Fd}", name=f"bnd{Fd}")
                nc.vector.tensor_scalar(
                    out=d[:], in0=src_ap, scalar1=mu5[:], scalar2=None, op0=ALU.subtract
                )
                sq5 = pp5.tile([Fd, G], f32, tag=f"bnsq{Fd}", name=f"bnsq{Fd}")
                nc.vector.tensor_tensor(out=sq5[:], in0=d[:], in1=d[:], op=ALU.mult)
                v = pp5.tile([Fd, 1], f32, tag=f"bnv{Fd}", name=f"bnv{Fd}")
                nc.vector.tensor_reduce(out=v[:], in_=sq5[:], axis=mybir.AxisListType.X, op=ALU.add)
                nc.vector.tensor_scalar(
                    out=v[:], in0=v[:], scalar1=1.0 / G, scalar2=EPS, op0=ALU.mult, op1=ALU.add
                )
                sd5 = pp5.tile([Fd, 1], f32, tag=f"bnsd{Fd}", name=f"bnsd{Fd}")
                nc.scalar.sqrt(out=sd5[:], in_=v[:])
                rs5 = pp5.tile([Fd, 1], f32, tag=f"bnrs{Fd}", name=f"bnrs{Fd}")
                nc.vector.reciprocal(out=rs5[:], in_=sd5[:])
                sc5 = pp5.tile([Fd, 1], f32, tag=f"bnsc{Fd}", name=f"bnsc{Fd}")
                nc.vector.tensor_tensor(out=sc5[:], in0=gl, in1=rs5[:], op=ALU.mult)
                nc.vector.tensor_scalar(
                    out=dest, in0=d[:], scalar1=sc5[:], scalar2=bl, op0=ALU.mult, op1=ALU.add
                )

            bn1 = pp5.tile([16, G], f32, tag="bn1")
            bn_t(xeT[:], 16, cols_s["gl1"][:], cols_s["bl1"][:], bn1[:])
            z1p = ps5.tile([16, G], f32, tag="z1p")
            nc.tensor.matmul(out=z1p[:], lhsT=Ws_s["W1"][:], rhs=bn1[:], start=True, stop=True)
            zs1 = pp5.tile([16, G], f32, tag="zs1")
            nc.scalar.activation(out=zs1[:], in_=z1p[:], func=AF.Relu, bias=cols_s["bW1"][:], scale=1.0)
            bn2a = pp5.tile([16, G], f32, tag="bn2a")
            bn_t(zs1[:], 16, cols_s["gl2a"][:], cols_s["bl2a"][:], bn2a[:])
            bn2b = pp5.tile([16, G], f32, tag="bn2b")
            bn_t(xeT[:], 16, cols_s["gl2b"][:], cols_s["bl2b"][:], bn2b[:])
            z2p = ps5.tile([16, G], f32, tag="z2p")
            nc.tensor.matmul(out=z2p[:], lhsT=Ws_s["W2a"][:], rhs=bn2a[:], start=True, stop=False)
            nc.tensor.matmul(out=z2p[:], lhsT=Ws_s["W2b"][:], rhs=bn2b[:], start=False, stop=True)
            zs2 = pp5.tile([16, G], f32, tag="zs2")
            nc.scalar.activation(out=zs2[:], in_=z2p[:], func=AF.Relu, bias=cols_s["bW2"][:], scale=1.0)
            bn3a = pp5.tile([16, G], f32, tag="bn3a")
            bn_t(zs2[:], 16, cols_s["gl3a"][:], cols_s["bl3a"][:], bn3a[:])
            bn3b = pp5.tile([16, G], f32, tag="bn3b")
            bn_t(xeT[:], 16, cols_s["gl3b"][:], cols_s["bl3b"][:], bn3b[:])
            z3p = ps5.tile([16, G], f32, tag="z3p")
            nc.tensor.matmul(out=z3p[:], lhsT=Ws_s["W3a"][:], rhs=bn3a[:], start=True, stop=False)
            nc.tensor.matmul(out=z3p[:], lhsT=Ws_s["W3b"][:], rhs=bn3b[:], start=False, stop=True)
            z3 = pp5.tile([16, G], f32, tag="z3")
            nc.scalar.activation(out=z3[:], in_=z3p[:], func=AF.Relu, bias=cols_s["bW3"][:], scale=1.0)
            ofp = ps5.tile([1, G], f32, tag="ofp")
            nc.tensor.matmul(out=ofp[:], lhsT=Ws_s["Wf"][:], rhs=z3[:], start=True, stop=True)
            ofs = pp5.tile([1, G], f32, tag="ofs")
            nc.vector.tensor_scalar(
                out=ofs[:], in0=ofp[:], scalar1=cols_s["bWf"][:], scalar2=None, op0=ALU.add
            )
            nc.sync.dma_start(out=out_t.ap(), in_=ofs[:])

    nc.compile()
    return nc


def run(inputs, n_cores=8, G=256, cfg_overrides=None, trace=False, enable_asserts=False):
    from concourse.bass_utils import run_bass_kernel_spmd

    in_maps, cfg = build_host_data(
        inputs["x"], inputs["edge_index"], inputs["batch"], n_cores, G
    )
    if cfg_overrides:
        cfg.update(cfg_overrides)
    add_weights(in_maps, inputs)
    nc = build_program(cfg, enable_asserts=enable_asserts)
    res = run_bass_kernel_spmd(nc, in_maps, core_ids=list(range(n_cores)), trace=trace)
    out = res.results[0]["out"].reshape(G, 1)
    return out, res, cfg


def kernel(**inputs):
    """Full inputs -> full [256, 1] output. Shards internally across 8 cores."""
    out, _, _ = run(inputs, n_cores=8, G=256)
    return np.asarray(out, np.float32)
